# revision 12
# baseline (speedup 1.0000x reference)
"""BiLSTM-CRF loss kernel for Trainium2 (8 NeuronCores, Bass/Tile).

Strategy (v2)
-------------
Cores 0-3 run the FORWARD LSTM direction, cores 4-7 the BACKWARD direction
(fed time-reversed x), each over 16 of the 64 sequences (data-parallel over
batch within each direction).  Pair (c, c+4) handles the same 16 sequences.

Per core, one fused chunked loop (CH timesteps per chunk):
  - GX chunk n+1 (x @ W_ih^T + bias) is computed into SBUF (bf16),
    interleaved with the recurrence steps of chunk n so the big matmuls
    fill tensor-engine bubbles left by the serial LSTM chain.
  - LSTM cell per step: gate order f,i,g,o; W_hh matmuls accumulate into
    PSUM, GX is added by identity-stationary matmuls per gate block so the
    activations read PSUM directly and start early (sigmoid(f,i) under the
    g/o matmuls).  h is written bf16 straight into an SBUF history buffer.
  - em partials per chunk from the SBUF h history; two masked slots are
    written to DRAM and pair-AllReduced (fwd slot / time-reversed bwd slot).
  - CRF: gold score via one-hot matmul reductions; partition function via
    probability-domain scan with a constant e^-kappa prescale folded into
    exp(em), two interleaved batch groups to hide semaphore latency, and a
    proper rescale only every R steps.  Final loss AllReduce over 8 cores.
"""

import sys

sys.path.insert(0, "/opt/trn_rl_repo")

import numpy as np
import ml_dtypes
from contextlib import ExitStack

import concourse.bass as bass
import concourse.bacc as bacc
import concourse.tile as tile
import concourse.mybir as mybir

F32 = mybir.dt.float32
BF16 = mybir.dt.bfloat16
I32 = mybir.dt.int32
AFT = mybir.ActivationFunctionType
ALU = mybir.AluOpType
AXL = mybir.AxisListType

NCORES = 8
NPAIR = 4  # fwd cores 0..3, bwd cores 4..7
KAPPA = 2.2  # CRF scan prescale: eem = exp(em - KAPPA)


# ---------------------------------------------------------------------------
# program builder (SPMD: one program, per-core divergence is data only)
# ---------------------------------------------------------------------------

def build_program(b, S, E, HD, T, B_full, CH=32, R=64, stop_after=None):
    """b: sequences per core; returns the Bass program."""
    KE = E // 128          # input-proj K tiles
    NH = HD // 128         # hidden K tiles (= h tiles)
    NM = 4 * NH            # gate m-tiles (permuted order f,i,g,o)
    SB = S * b             # (t, b) flattened column count
    W = NH * b             # per-step h column width  (64)
    SBc = CH * b           # columns per chunk         (512)
    NCHK = S // CH
    assert S % CH == 0 and CH % 2 == 0 and NM == 16 and CH >= 2 * NM // 2

    nc = bacc.Bacc("TRN2", target_bir_lowering=False, debug=False,
                   num_devices=NCORES)

    # ---- I/O ----
    xT = nc.dram_tensor("xT", [KE, 128, SB], BF16, kind="ExternalInput")
    wihT = nc.dram_tensor("wihT", [KE, 128, 4 * HD], BF16, kind="ExternalInput")
    whhT = nc.dram_tensor("whhT", [NH, 128, 4 * HD], BF16, kind="ExternalInput")
    bias4 = nc.dram_tensor("bias4", [128, NM], F32, kind="ExternalInput")
    ident = nc.dram_tensor("ident", [128, 128], BF16, kind="ExternalInput")
    wtagT = nc.dram_tensor("wtagT", [NH, 128, T], BF16, kind="ExternalInput")
    tagb = nc.dram_tensor("tagb", [T, 1], F32, kind="ExternalInput")
    m0 = nc.dram_tensor("m0", [T, 1], F32, kind="ExternalInput")
    m1 = nc.dram_tensor("m1", [T, 1], F32, kind="ExternalInput")
    labT = nc.dram_tensor("labT", [S, b], I32, kind="ExternalInput")
    transm = nc.dram_tensor("transm", [T, T], F32, kind="ExternalInput")
    startv = nc.dram_tensor("startv", [T, 1], F32, kind="ExternalInput")
    endv = nc.dram_tensor("endv", [T, 1], F32, kind="ExternalInput")
    loss = nc.dram_tensor("loss", [1, 1], F32, kind="ExternalOutput")

    with tile.TileContext(nc) as tc, ExitStack() as top:
        dram = top.enter_context(tc.tile_pool(name="dram", bufs=1, space="DRAM"))
        emdb = dram.tile([2, T, SB], BF16)
        emdbo = dram.tile([2, T, SB], BF16)
        lossdb = dram.tile([1, 1], F32)
        lossout = dram.tile([1, 1], F32)

        # ============== fused phase A+B+C (chunked) ==============
        ab = ExitStack()
        persist = ab.enter_context(tc.tile_pool(name="persist", bufs=1))
        hist = persist.tile([128, S * W], BF16)      # h history [t, k, b]
        c_sb = persist.tile([128, W], F32)
        wp = ab.enter_context(tc.tile_pool(name="weights", bufs=1))
        wih_sb = wp.tile([128, KE * 4 * HD], BF16)
        whh_sb = wp.tile([128, NH * 4 * HD], BF16)
        bias_sb = wp.tile([128, NM], F32)
        ident_sb = wp.tile([128, 128], BF16)
        wtag_sb = wp.tile([128, NH * T], BF16)
        tagb_sb = wp.tile([T, 1], F32)
        m0_sb = wp.tile([T, 1], F32)
        m1_sb = wp.tile([T, 1], F32)
        nc.sync.dma_start(wih_sb[:], wihT[:])
        nc.sync.dma_start(whh_sb[:], whhT[:])
        nc.sync.dma_start(bias_sb[:], bias4[:])
        nc.sync.dma_start(ident_sb[:], ident[:])
        nc.sync.dma_start(wtag_sb[:], wtagT[:])
        nc.sync.dma_start(tagb_sb[:], tagb[:])
        nc.sync.dma_start(m0_sb[:], m0[:])
        nc.sync.dma_start(m1_sb[:], m1[:])

        xp = ab.enter_context(tc.tile_pool(name="xin", bufs=2))
        gxsp = ab.enter_context(tc.tile_pool(name="gxs", bufs=2))
        gxps = ab.enter_context(tc.tile_pool(name="gxps", bufs=2, space="PSUM"))
        rp = ab.enter_context(tc.tile_pool(name="recps", bufs=2, space="PSUM"))
        ep = ab.enter_context(tc.tile_pool(name="emps", bufs=2, space="PSUM"))
        tp = ab.enter_context(tc.tile_pool(name="steptmp", bufs=2))
        sp2 = ab.enter_context(tc.tile_pool(name="emtmp", bufs=2))

        def emit_gx_mtile(m, xt_sb, gxc):
            # gxc layout: [128, (tt, m, b)] — per-step gx blocks contiguous
            ps = gxps.tile([128, SBc], F32)
            for k in range(KE):
                nc.tensor.matmul(
                    ps[:],
                    wih_sb[:, k * 4 * HD + m * 128:k * 4 * HD + (m + 1) * 128],
                    xt_sb[:, k * SBc:(k + 1) * SBc],
                    start=(k == 0), stop=(k == KE - 1))
            out_ap = gxc[:].rearrange("p (t m c) -> p m t c", m=NM, c=b)[:, m]
            nc.vector.tensor_scalar(out_ap, ps[:].rearrange(
                "p (t c) -> p t c", c=b), bias_sb[:, m:m + 1], None, op0=ALU.add)

        # prologue: x + GX for chunk 0
        xt_sb = xp.tile([128, KE * SBc], BF16)
        nc.sync.dma_start(xt_sb[:], xT[:, :, 0:SBc])
        gxc = gxsp.tile([128, NM * SBc], BF16)
        for m in range(NM):
            emit_gx_mtile(m, xt_sb, gxc)

        for n in range(NCHK):
            gx_cur = gxc
            if n + 1 < NCHK:
                xt_sb = xp.tile([128, KE * SBc], BF16)
                nc.sync.dma_start(
                    xt_sb[:], xT[:, :, (n + 1) * SBc:(n + 2) * SBc])
                gxc = gxsp.tile([128, NM * SBc], BF16)

            for tt in range(CH):
                t = n * CH + tt

                def gx_ap(mlo, mn):
                    # contiguous [128, mn*b] slice of step tt's gx block
                    return gx_cur[:, tt * NM * b + mlo * b:
                                  tt * NM * b + (mlo + mn) * b]

                if t == 0:
                    sig = tp.tile([128, 3 * W], F32, tag="sig")
                    nc.scalar.activation(sig[:, 0:2 * W], gx_ap(0, 2 * NH),
                                         AFT.Sigmoid)
                    tg = tp.tile([128, W], F32, tag="tg")
                    nc.scalar.activation(tg[:], gx_ap(2 * NH, NH), AFT.Tanh)
                    nc.scalar.activation(sig[:, 2 * W:3 * W], gx_ap(3 * NH, NH),
                                         AFT.Sigmoid)
                    nc.vector.tensor_mul(c_sb[:], sig[:, W:2 * W], tg[:])
                else:
                    h_prev = hist[:, (t - 1) * W:t * W]
                    ps = rp.tile([128, NM * b], F32)

                    def cell_block(mlo, mn):
                        for mm in range(mlo, mlo + mn):
                            for kt in range(NH):
                                nc.tensor.matmul(
                                    ps[:, mm * b:(mm + 1) * b],
                                    whh_sb[:, kt * 4 * HD + mm * 128:
                                           kt * 4 * HD + (mm + 1) * 128],
                                    h_prev[:, kt * b:(kt + 1) * b],
                                    start=(kt == 0), stop=(kt == NH - 1))
                        nc.tensor.matmul(
                            ps[:, mlo * b:(mlo + mn) * b],
                            ident_sb[:], gx_ap(mlo, mn),
                            start=False, stop=True, skip_group_check=True)

                    cell_block(0, 2 * NH)         # f, i
                    sig = tp.tile([128, 3 * W], F32, tag="sig")
                    nc.scalar.activation(sig[:, 0:2 * W], ps[:, 0:2 * W],
                                         AFT.Sigmoid)
                    cell_block(2 * NH, NH)        # g
                    tg = tp.tile([128, W], F32, tag="tg")
                    nc.scalar.activation(tg[:], ps[:, 2 * W:3 * W], AFT.Tanh)
                    cell_block(3 * NH, NH)        # o
                    t1 = tp.tile([128, W], F32, tag="t1")
                    nc.vector.tensor_mul(t1[:], sig[:, 0:W], c_sb[:])
                    t2 = tp.tile([128, W], F32, tag="t2")
                    nc.vector.tensor_mul(t2[:], sig[:, W:2 * W], tg[:])
                    nc.vector.tensor_add(c_sb[:], t1[:], t2[:])
                    nc.scalar.activation(sig[:, 2 * W:3 * W], ps[:, 3 * W:4 * W],
                                         AFT.Sigmoid)
                tanc = tp.tile([128, W], F32, tag="tanc")
                nc.scalar.activation(tanc[:], c_sb[:], AFT.Tanh)
                nc.vector.tensor_mul(hist[:, t * W:(t + 1) * W],
                                     sig[:, 2 * W:3 * W], tanc[:])

                # interleave GX production for chunk n+1 into this chunk
                if n + 1 < NCHK and tt % 2 == 1 and tt // 2 < NM:
                    emit_gx_mtile(tt // 2, xt_sb, gxc)

            # ---- em partial for chunk n ----
            hv = hist[:, n * CH * W:(n + 1) * CH * W].rearrange(
                "p (t k c) -> p t k c", t=CH, k=NH)
            pse = ep.tile([T, SBc], F32)
            for kt in range(NH):
                nc.tensor.matmul(
                    pse[:].rearrange("p (t c) -> p t c", t=CH),
                    wtag_sb[:, kt * T:(kt + 1) * T],
                    hv[:, :, kt, :],
                    start=(kt == 0), stop=(kt == NH - 1))
            s0 = sp2.tile([T, SBc], BF16, tag="s0")
            nc.vector.tensor_scalar(s0[:], pse[:], tagb_sb[:], m0_sb[:],
                                    op0=ALU.add, op1=ALU.mult)
            nc.sync.dma_start(emdb[0, :, n * SBc:(n + 1) * SBc], s0[:])
            s1 = sp2.tile([T, SBc], BF16, tag="s1")
            nc.vector.tensor_scalar(s1[:], pse[:], tagb_sb[:], m1_sb[:],
                                    op0=ALU.add, op1=ALU.mult)
            nc.sync.dma_start(emdb[1, :, n * SBc:(n + 1) * SBc], s1[:])

        if stop_after == 'B':
            with tc.tile_pool(name="bail", bufs=1) as bp:
                bt = bp.tile([1, 1], F32)
                nc.vector.tensor_copy(bt[:], c_sb[0:1, 0:1])
                nc.sync.dma_start(loss[:], bt[:])
            ab.close()
            nc.compile()
            return nc

        ab.close()

        # ---- pair AllReduce of em slots ----
        nc.gpsimd.collective_compute(
            "AllReduce", ALU.add,
            replica_groups=[[c, c + NPAIR] for c in range(NPAIR)],
            ins=[emdb.opt()], outs=[emdbo.opt()])

        crf = top.enter_context(tc.tile_pool(name="crf", bufs=1))
        em_full = crf.tile([T, SB], F32, tag="emfull")
        eem = crf.tile([T, SB], F32, tag="eem")
        e0b = crf.tile([T, SB], BF16, tag="e0b")
        e1b = crf.tile([T, SB], BF16, tag="e1b")
        nc.sync.dma_start(e0b[:], emdbo[0])
        nc.sync.dma_start(
            e1b[:], emdbo[1].rearrange("j (t c) -> j t c", t=S)[:, ::-1, :])
        nc.vector.tensor_add(em_full[:], e0b[:], e1b[:])

        if stop_after == 'C':
            with tc.tile_pool(name="bail", bufs=1) as bp:
                bt = bp.tile([1, 1], F32)
                nc.vector.tensor_copy(bt[:], em_full[0:1, 0:1])
                nc.sync.dma_start(loss[:], bt[:])
            nc.compile()
            return nc

        # ---------------- Phase D: CRF ----------------
        with ExitStack() as ph:
            sp = ph.enter_context(tc.tile_pool(name="crftmp", bufs=2))
            big = ph.enter_context(tc.tile_pool(name="crfbig", bufs=2))
            ap_ = ph.enter_context(tc.tile_pool(name="alphas", bufs=2))

            cst = sp.tile([T, T], F32, tag="cst")        # transitions
            nc.sync.dma_start(cst[:], transm[:])
            st_sb = sp.tile([T, 1], F32, tag="stv")
            nc.sync.dma_start(st_sb[:], startv[:])
            en_sb = sp.tile([T, 1], F32, tag="env")
            nc.sync.dma_start(en_sb[:], endv[:])

            # --- numerator ---
            lab9 = big.tile([T, SB], I32, tag="big")
            nc.sync.dma_start(
                lab9[:],
                labT[:].rearrange("s c -> (s c)")[None, :].broadcast_to((T, SB)))
            labf = big.tile([T, SB], F32, tag="big")
            nc.vector.tensor_copy(labf[:], lab9[:])
            io9 = sp.tile([T, 1], I32, tag="io9")
            nc.gpsimd.iota(io9[:], pattern=[[0, 1]], base=0, channel_multiplier=1)
            io9f = sp.tile([T, 1], F32, tag="io9f")
            nc.vector.tensor_copy(io9f[:], io9[:])
            onehot = big.tile([T, SB], F32, tag="big")
            nc.vector.tensor_scalar(onehot[:], labf[:], io9f[:], None,
                                    op0=ALU.is_equal)
            gmul = big.tile([T, SB], F32, tag="big")
            nc.vector.tensor_mul(gmul[:], onehot[:], em_full[:])
            acc = sp.tile([T, b], F32, tag="acc")
            nc.vector.tensor_reduce(
                acc[:], gmul[:].rearrange("j (t c) -> j c t", c=b),
                op=ALU.add, axis=AXL.X)
            # start/end gold scores
            stsc = sp.tile([T, b], F32, tag="stsc")
            nc.vector.tensor_scalar_mul(stsc[:], onehot[:, 0:b], st_sb[:])
            nc.vector.tensor_add(acc[:], acc[:], stsc[:])
            ensc = sp.tile([T, b], F32, tag="ensc")
            nc.vector.tensor_scalar_mul(ensc[:], onehot[:, (S - 1) * b:S * b],
                                        en_sb[:])
            nc.vector.tensor_add(acc[:], acc[:], ensc[:])
            # transition gold scores: TH = T^T @ onehot ; V = TH * onehot_next
            numps = ExitStack()
            pp = numps.enter_context(
                tc.tile_pool(name="numps", bufs=2, space="PSUM"))
            for tc0 in range(0, S - 1, 32):
                tn = min(32, S - 1 - tc0)
                thp = pp.tile([T, 32 * b], F32, tag="thp")
                nc.tensor.matmul(thp[:, 0:tn * b], cst[:],
                                 onehot[:, tc0 * b:(tc0 + tn) * b],
                                 start=True, stop=True)
                v = sp.tile([T, 32 * b], F32, tag="v")
                nc.vector.tensor_mul(v[:, 0:tn * b], thp[:, 0:tn * b],
                                     onehot[:, (tc0 + 1) * b:(tc0 + 1 + tn) * b])
                vr = sp.tile([T, b], F32, tag="vr")
                nc.vector.tensor_reduce(
                    vr[:], v[:, 0:tn * b].rearrange("j (t c) -> j c t", c=b),
                    op=ALU.add, axis=AXL.X)
                nc.vector.tensor_add(acc[:], acc[:], vr[:])
            ones9 = sp.tile([T, 1], F32, tag="ones9")
            nc.vector.memset(ones9[:], 1.0)
            ones19 = sp.tile([1, T], F32, tag="ones19")
            nc.vector.memset(ones19[:], 1.0)
            nump = pp.tile([1, b], F32, tag="nump")
            nc.tensor.matmul(nump[:], ones9[:], acc[:], start=True, stop=True)
            num_sb = sp.tile([1, b], F32, tag="num")
            nc.vector.tensor_copy(num_sb[:], nump[:])
            numps.close()
            pp = ph.enter_context(tc.tile_pool(name="scanps", bufs=2, space="PSUM"))
            pp2 = ph.enter_context(tc.tile_pool(name="scanps2", bufs=2, space="PSUM"))

            # --- partition function (probability-domain scan, prescaled) ---
            NG = 2                     # interleaved batch groups
            gb = b // NG               # 8 sequences per group
            Em = sp.tile([T, T], F32, tag="Em")
            nc.scalar.activation(Em[:], cst[:], AFT.Exp)
            kneg = sp.tile([T, 1], F32, tag="kneg")
            nc.vector.memset(kneg[:], -KAPPA)
            nc.scalar.activation(eem[:], em_full[:], AFT.Exp, bias=kneg[:])
            es = sp.tile([T, 1], F32, tag="es")
            nc.scalar.activation(es[:], st_sb[:], AFT.Exp)
            ee = sp.tile([T, 1], F32, tag="ee")
            nc.scalar.activation(ee[:], en_sb[:], AFT.Exp)
            logacc = sp.tile([1, b], F32, tag="logacc")
            nc.vector.memset(logacc[:], 0.0)
            alphas = []
            for g in range(NG):
                al = ap_.tile([T, gb], F32, tag=f"al{g}")
                nc.vector.tensor_scalar_mul(al[:], eem[:, g * gb:(g + 1) * gb],
                                            es[:])
                alphas.append(al)
            for t in range(1, S):
                apss = []
                for g in range(NG):
                    aps = pp.tile([T, gb], F32, tag=f"aps{g}")
                    nc.tensor.matmul(aps[:], Em[:], alphas[g][:],
                                     start=True, stop=True)
                    apss.append(aps)
                for g in range(NG):
                    al = ap_.tile([T, gb], F32, tag=f"al{g}")
                    nc.vector.tensor_mul(
                        al[:], apss[g][:],
                        eem[:, t * b + g * gb:t * b + (g + 1) * gb])
                    alphas[g] = al
                if t % R == 0:
                    for g in range(NG):
                        ssum = pp2.tile([1, gb], F32, tag="ssum")
                        nc.tensor.matmul(ssum[:], ones9[:], alphas[g][:],
                                         start=True, stop=True)
                        ls = sp.tile([1, gb], F32, tag=f"ls{g}")
                        nc.scalar.activation(ls[:], ssum[:], AFT.Ln)
                        nc.vector.tensor_add(
                            logacc[:, g * gb:(g + 1) * gb],
                            logacc[:, g * gb:(g + 1) * gb], ls[:])
                        rc = sp.tile([1, gb], F32, tag=f"rc{g}")
                        nc.vector.reciprocal(rc[:], ssum[:])
                        bc = pp2.tile([T, gb], F32, tag="bc")
                        nc.tensor.matmul(bc[:], ones19[:], rc[:],
                                         start=True, stop=True)
                        al = ap_.tile([T, gb], F32, tag=f"al{g}")
                        nc.vector.tensor_mul(al[:], alphas[g][:], bc[:])
                        alphas[g] = al
            lv = sp.tile([1, b], F32, tag="lv")
            for g in range(NG):
                zp = pp.tile([1, gb], F32, tag=f"aps{g}")
                nc.tensor.matmul(zp[:], ee[:], alphas[g][:],
                                 start=True, stop=True)
                lz = sp.tile([1, gb], F32, tag=f"lz{g}")
                nc.scalar.activation(lz[:], zp[:], AFT.Ln)
                logz = sp.tile([1, gb], F32, tag=f"logz{g}")
                nc.vector.tensor_add(logz[:], lz[:],
                                     logacc[:, g * gb:(g + 1) * gb])
                # num - (logz + S*kappa)
                nc.vector.tensor_sub(lv[:, g * gb:(g + 1) * gb],
                                     num_sb[:, g * gb:(g + 1) * gb], logz[:])
            lvk = sp.tile([1, b], F32, tag="lvk")
            nc.vector.tensor_scalar_add(lvk[:], lv[:], -float(S) * KAPPA)
            tot = sp.tile([1, 1], F32, tag="tot")
            nc.vector.tensor_reduce(tot[:], lvk[:], op=ALU.add, axis=AXL.X)
            sc = sp.tile([1, 1], F32, tag="sc")
            nc.vector.tensor_scalar_mul(sc[:], tot[:], -1.0 / (2.0 * B_full))
            nc.sync.dma_start(lossdb[:], sc[:])
            nc.gpsimd.collective_compute(
                "AllReduce", ALU.add,
                replica_groups=[list(range(NCORES))],
                ins=[lossdb.opt()], outs=[lossout.opt()])
            lf = sp.tile([1, 1], F32, tag="lf")
            nc.sync.dma_start(lf[:], lossout[:])
            nc.sync.dma_start(loss[:], lf[:])

    nc.compile()
    return nc


# ---------------------------------------------------------------------------
# host-side sharding
# ---------------------------------------------------------------------------

def _perm_figo(HD):
    # torch gate order i,f,g,o -> f,i,g,o
    return np.concatenate([
        np.arange(HD, 2 * HD), np.arange(0, HD),
        np.arange(2 * HD, 3 * HD), np.arange(3 * HD, 4 * HD)])


def shard_inputs(inputs, b, S, E, HD, T):
    KE, NH = E // 128, HD // 128
    perm = _perm_figo(HD)
    bf = ml_dtypes.bfloat16
    x = np.asarray(inputs["x"], np.float32)
    labels = np.asarray(inputs["labels"]).astype(np.int32)
    trans = np.asarray(inputs["transitions"], np.float32)
    startv = np.asarray(inputs["start_trans"], np.float32).reshape(T, 1)
    endv = np.asarray(inputs["end_trans"], np.float32).reshape(T, 1)
    Wtag = np.asarray(inputs["W_tag"], np.float32)
    btag = np.asarray(inputs["b_tag"], np.float32).reshape(T, 1)
    identm = np.eye(128, dtype=np.float32).astype(bf)

    per_dir = {}
    for d, sfx in enumerate(("f", "b")):
        Wih = np.asarray(inputs[f"W_ih_{sfx}"], np.float32)[perm]
        Whh = np.asarray(inputs[f"W_hh_{sfx}"], np.float32)[perm]
        bias = (np.asarray(inputs[f"b_ih_{sfx}"], np.float32)
                + np.asarray(inputs[f"b_hh_{sfx}"], np.float32))[perm]
        per_dir[d] = dict(
            wihT=np.ascontiguousarray(
                Wih.T.reshape(KE, 128, 4 * HD)).astype(bf),
            whhT=np.ascontiguousarray(
                Whh.T.reshape(NH, 128, 4 * HD)).astype(bf),
            bias4=np.ascontiguousarray(
                bias.reshape(4 * NH, 128).T).astype(np.float32),
            wtagT=np.ascontiguousarray(
                Wtag[:, d * HD:(d + 1) * HD].T.reshape(NH, 128, T)).astype(bf),
            tagb=btag if d == 0 else np.zeros_like(btag),
            m0=np.full((T, 1), 1.0 - d, np.float32),
            m1=np.full((T, 1), float(d), np.float32),
        )

    in_maps = []
    for c in range(NCORES):
        d = c // NPAIR                      # 0 fwd, 1 bwd
        g = c % NPAIR                       # batch group
        xs = x[g * b:(g + 1) * b]           # (b, S, E)
        if d == 1:
            xs = xs[:, ::-1, :]
        xTc = np.ascontiguousarray(xs.transpose(2, 1, 0).reshape(KE, 128, S * b)
                                   ).astype(bf)
        m = dict(per_dir[d])
        m["xT"] = xTc
        m["labT"] = np.ascontiguousarray(labels[g * b:(g + 1) * b].T)
        m["transm"] = trans
        m["startv"] = startv
        m["endv"] = endv
        m["ident"] = identm
        in_maps.append(m)
    return in_maps


# ---------------------------------------------------------------------------
# entry point
# ---------------------------------------------------------------------------

_B, _S, _E, _HD, _T = 64, 512, 1024, 512, 9
_cache = {}


def _get_program():
    if "nc" not in _cache:
        _cache["nc"] = build_program(_B // NPAIR, _S, _E, _HD, _T, _B)
    return _cache["nc"]


def kernel(**inputs) -> np.ndarray:
    from concourse.bass_utils import run_bass_kernel_spmd
    nc = _get_program()
    in_maps = shard_inputs(inputs, _B // NPAIR, _S, _E, _HD, _T)
    res = run_bass_kernel_spmd(nc, in_maps, list(range(NCORES)))
    out = np.asarray(res.results[0]["loss"], np.float32).reshape(())
    return out


# revision 14
# speedup vs baseline: 1.1685x; 1.1685x over previous
"""BiLSTM-CRF loss kernel for Trainium2 (8 NeuronCores, Bass/Tile).

Strategy (v2)
-------------
Cores 0-3 run the FORWARD LSTM direction, cores 4-7 the BACKWARD direction
(fed time-reversed x), each over 16 of the 64 sequences (data-parallel over
batch within each direction).  Pair (c, c+4) handles the same 16 sequences.

Per core, one fused chunked loop (CH timesteps per chunk):
  - GX chunk n+1 (x @ W_ih^T + bias) is computed into SBUF (bf16),
    interleaved with the recurrence steps of chunk n so the big matmuls
    fill tensor-engine bubbles left by the serial LSTM chain.
  - LSTM cell per step: gate order f,i,g,o; W_hh matmuls accumulate into
    PSUM, GX is added by identity-stationary matmuls per gate block so the
    activations read PSUM directly and start early (sigmoid(f,i) under the
    g/o matmuls).  h is written bf16 straight into an SBUF history buffer.
  - em partials per chunk from the SBUF h history; two masked slots are
    written to DRAM and pair-AllReduced (fwd slot / time-reversed bwd slot).
  - CRF: gold score via one-hot matmul reductions; partition function via
    probability-domain scan with a constant e^-kappa prescale folded into
    exp(em), two interleaved batch groups to hide semaphore latency, and a
    proper rescale only every R steps.  Final loss AllReduce over 8 cores.
"""

import sys

sys.path.insert(0, "/opt/trn_rl_repo")

import numpy as np
import ml_dtypes
from contextlib import ExitStack

import concourse.bass as bass
import concourse.bacc as bacc
import concourse.tile as tile
import concourse.mybir as mybir

F32 = mybir.dt.float32
BF16 = mybir.dt.bfloat16
I32 = mybir.dt.int32
AFT = mybir.ActivationFunctionType
ALU = mybir.AluOpType
AXL = mybir.AxisListType

NCORES = 8
NPAIR = 4  # fwd cores 0..3, bwd cores 4..7
KAPPA = 2.2  # CRF scan prescale: eem = exp(em - KAPPA)


# ---------------------------------------------------------------------------
# program builder (SPMD: one program, per-core divergence is data only)
# ---------------------------------------------------------------------------

def build_program(b, S, E, HD, T, B_full, CH=32, R=64, stop_after=None):
    """b: sequences per core; returns the Bass program."""
    KE = E // 128          # input-proj K tiles
    NH = HD // 128         # hidden K tiles (= h tiles)
    NM = 4 * NH            # gate m-tiles (permuted order f,i,g,o)
    SB = S * b             # (t, b) flattened column count
    W = NH * b             # per-step h column width  (64)
    SBc = CH * b           # columns per chunk         (512)
    NCHK = S // CH
    assert S % CH == 0 and CH % 2 == 0 and NM == 16 and CH >= 2 * NM // 2

    nc = bacc.Bacc("TRN2", target_bir_lowering=False, debug=False,
                   num_devices=NCORES)

    # ---- I/O ----
    xT = nc.dram_tensor("xT", [KE, 128, SB], BF16, kind="ExternalInput")
    wihT = nc.dram_tensor("wihT", [KE, 128, 4 * HD], BF16, kind="ExternalInput")
    whhT = nc.dram_tensor("whhT", [NH, 128, 4 * HD], BF16, kind="ExternalInput")
    bias4 = nc.dram_tensor("bias4", [128, NM], F32, kind="ExternalInput")
    ident = nc.dram_tensor("ident", [128, 128], BF16, kind="ExternalInput")
    wtagT = nc.dram_tensor("wtagT", [NH, 128, T], BF16, kind="ExternalInput")
    tagb = nc.dram_tensor("tagb", [T, 1], F32, kind="ExternalInput")
    m0 = nc.dram_tensor("m0", [T, 1], F32, kind="ExternalInput")
    m1 = nc.dram_tensor("m1", [T, 1], F32, kind="ExternalInput")
    labT = nc.dram_tensor("labT", [S, b], I32, kind="ExternalInput")
    transm = nc.dram_tensor("transm", [T, T], F32, kind="ExternalInput")
    startv = nc.dram_tensor("startv", [T, 1], F32, kind="ExternalInput")
    endv = nc.dram_tensor("endv", [T, 1], F32, kind="ExternalInput")
    loss = nc.dram_tensor("loss", [1, 1], F32, kind="ExternalOutput")

    with tile.TileContext(nc) as tc, ExitStack() as top:
        dram = top.enter_context(tc.tile_pool(name="dram", bufs=1, space="DRAM"))
        emdb = dram.tile([2, T, SB], BF16)
        emdbo = dram.tile([2, T, SB], BF16)
        lossdb = dram.tile([1, 1], F32)
        lossout = dram.tile([1, 1], F32)

        # ============== fused phase A+B+C (chunked) ==============
        ab = ExitStack()
        persist = ab.enter_context(tc.tile_pool(name="persist", bufs=1))
        hist = persist.tile([128, S * W], BF16)      # h history [t, k, b]
        c_sb = persist.tile([128, W], F32)
        wp = ab.enter_context(tc.tile_pool(name="weights", bufs=1))
        wih_sb = wp.tile([128, KE * 4 * HD], BF16)
        whh_sb = wp.tile([128, NH * 4 * HD], BF16)
        bias_sb = wp.tile([128, NM], F32)
        ident_sb = wp.tile([128, 128], BF16)
        wtag_sb = wp.tile([128, NH * T], BF16)
        tagb_sb = wp.tile([T, 1], F32)
        m0_sb = wp.tile([T, 1], F32)
        m1_sb = wp.tile([T, 1], F32)
        nc.sync.dma_start(wih_sb[:], wihT[:])
        nc.sync.dma_start(whh_sb[:], whhT[:])
        nc.sync.dma_start(bias_sb[:], bias4[:])
        nc.sync.dma_start(ident_sb[:], ident[:])
        nc.sync.dma_start(wtag_sb[:], wtagT[:])
        nc.sync.dma_start(tagb_sb[:], tagb[:])
        nc.sync.dma_start(m0_sb[:], m0[:])
        nc.sync.dma_start(m1_sb[:], m1[:])

        xp = ab.enter_context(tc.tile_pool(name="xin", bufs=2))
        gxsp = ab.enter_context(tc.tile_pool(name="gxs", bufs=2))
        gxps = ab.enter_context(tc.tile_pool(name="gxps", bufs=1, space="PSUM"))
        rp = ab.enter_context(tc.tile_pool(name="recps", bufs=2, space="PSUM"))
        ep = ab.enter_context(tc.tile_pool(name="emps", bufs=1, space="PSUM"))
        tp = ab.enter_context(tc.tile_pool(name="steptmp", bufs=2))
        sp2 = ab.enter_context(tc.tile_pool(name="emtmp", bufs=2))

        def emit_gx_mtile(m, xt_sb, gxc):
            # gxc layout: [128, (tt, m, b)] — per-step gx blocks contiguous
            ps = gxps.tile([128, SBc], F32)
            for k in range(KE):
                nc.tensor.matmul(
                    ps[:],
                    wih_sb[:, k * 4 * HD + m * 128:k * 4 * HD + (m + 1) * 128],
                    xt_sb[:, k * SBc:(k + 1) * SBc],
                    start=(k == 0), stop=(k == KE - 1))
            out_ap = gxc[:].rearrange("p (t m c) -> p m t c", m=NM, c=b)[:, m]
            nc.vector.tensor_scalar(out_ap, ps[:].rearrange(
                "p (t c) -> p t c", c=b), bias_sb[:, m:m + 1], None, op0=ALU.add)

        # prologue: x + GX for chunk 0
        xt_sb = xp.tile([128, KE * SBc], BF16)
        nc.sync.dma_start(xt_sb[:], xT[:, :, 0:SBc])
        gxc = gxsp.tile([128, NM * SBc], BF16)
        for m in range(NM):
            emit_gx_mtile(m, xt_sb, gxc)

        for n in range(NCHK):
            gx_cur = gxc
            if n + 1 < NCHK:
                xt_sb = xp.tile([128, KE * SBc], BF16)
                nc.sync.dma_start(
                    xt_sb[:], xT[:, :, (n + 1) * SBc:(n + 2) * SBc])
                gxc = gxsp.tile([128, NM * SBc], BF16)

            for tt in range(CH):
                t = n * CH + tt

                def gx_ap(mlo, mn):
                    # contiguous [128, mn*b] slice of step tt's gx block
                    return gx_cur[:, tt * NM * b + mlo * b:
                                  tt * NM * b + (mlo + mn) * b]

                if t == 0:
                    sig = tp.tile([128, 3 * W], F32, tag="sig")
                    nc.scalar.activation(sig[:, 0:2 * W], gx_ap(0, 2 * NH),
                                         AFT.Sigmoid)
                    tg = tp.tile([128, W], F32, tag="tg")
                    nc.scalar.activation(tg[:], gx_ap(2 * NH, NH), AFT.Tanh)
                    nc.scalar.activation(sig[:, 2 * W:3 * W], gx_ap(3 * NH, NH),
                                         AFT.Sigmoid)
                    nc.vector.tensor_mul(c_sb[:], sig[:, W:2 * W], tg[:])
                else:
                    h_prev = hist[:, (t - 1) * W:t * W]

                    def cell_block(pst, mlo, mn):
                        # own PSUM bank per gate block: start=True zeroing is
                        # bank-granular, so blocks must not share banks with
                        # regions still being read by the activations.
                        for mi in range(mn):
                            mm = mlo + mi
                            for kt in range(NH):
                                nc.tensor.matmul(
                                    pst[:, mi * b:(mi + 1) * b],
                                    whh_sb[:, kt * 4 * HD + mm * 128:
                                           kt * 4 * HD + (mm + 1) * 128],
                                    h_prev[:, kt * b:(kt + 1) * b],
                                    start=(kt == 0), stop=(kt == NH - 1))
                        nc.tensor.matmul(
                            pst[:, 0:mn * b],
                            ident_sb[:], gx_ap(mlo, mn),
                            start=False, stop=True, skip_group_check=True)

                    psfi = rp.tile([128, 2 * W], F32, tag="psfi")
                    cell_block(psfi, 0, 2 * NH)   # f, i
                    sig = tp.tile([128, 3 * W], F32, tag="sig")
                    nc.scalar.activation(sig[:, 0:2 * W], psfi[:], AFT.Sigmoid)
                    psg = rp.tile([128, W], F32, tag="psg")
                    cell_block(psg, 2 * NH, NH)   # g
                    tg = tp.tile([128, W], F32, tag="tg")
                    nc.scalar.activation(tg[:], psg[:], AFT.Tanh)
                    pso = rp.tile([128, W], F32, tag="pso")
                    cell_block(pso, 3 * NH, NH)   # o
                    t1 = tp.tile([128, W], F32, tag="t1")
                    nc.vector.tensor_mul(t1[:], sig[:, 0:W], c_sb[:])
                    t2 = tp.tile([128, W], F32, tag="t2")
                    nc.vector.tensor_mul(t2[:], sig[:, W:2 * W], tg[:])
                    nc.vector.tensor_add(c_sb[:], t1[:], t2[:])
                    nc.scalar.activation(sig[:, 2 * W:3 * W], pso[:],
                                         AFT.Sigmoid)
                tanc = tp.tile([128, W], F32, tag="tanc")
                nc.scalar.activation(tanc[:], c_sb[:], AFT.Tanh)
                nc.vector.tensor_mul(hist[:, t * W:(t + 1) * W],
                                     sig[:, 2 * W:3 * W], tanc[:])

                # interleave GX production for chunk n+1 into this chunk
                if n + 1 < NCHK and tt % 2 == 1 and tt // 2 < NM:
                    emit_gx_mtile(tt // 2, xt_sb, gxc)

            # ---- em partial for chunk n ----
            hv = hist[:, n * CH * W:(n + 1) * CH * W].rearrange(
                "p (t k c) -> p t k c", t=CH, k=NH)
            pse = ep.tile([T, SBc], F32)
            for kt in range(NH):
                nc.tensor.matmul(
                    pse[:].rearrange("p (t c) -> p t c", t=CH),
                    wtag_sb[:, kt * T:(kt + 1) * T],
                    hv[:, :, kt, :],
                    start=(kt == 0), stop=(kt == NH - 1))
            s0 = sp2.tile([T, SBc], BF16, tag="s0")
            nc.vector.tensor_scalar(s0[:], pse[:], tagb_sb[:], m0_sb[:],
                                    op0=ALU.add, op1=ALU.mult)
            nc.sync.dma_start(emdb[0, :, n * SBc:(n + 1) * SBc], s0[:])
            s1 = sp2.tile([T, SBc], BF16, tag="s1")
            nc.vector.tensor_scalar(s1[:], pse[:], tagb_sb[:], m1_sb[:],
                                    op0=ALU.add, op1=ALU.mult)
            nc.sync.dma_start(emdb[1, :, n * SBc:(n + 1) * SBc], s1[:])

        if stop_after == 'B':
            with tc.tile_pool(name="bail", bufs=1) as bp:
                bt = bp.tile([1, 1], F32)
                nc.vector.tensor_copy(bt[:], c_sb[0:1, 0:1])
                nc.sync.dma_start(loss[:], bt[:])
            ab.close()
            nc.compile()
            return nc

        ab.close()

        # ---- pair AllReduce of em slots ----
        nc.gpsimd.collective_compute(
            "AllReduce", ALU.add,
            replica_groups=[[c, c + NPAIR] for c in range(NPAIR)],
            ins=[emdb.opt()], outs=[emdbo.opt()])

        crf = top.enter_context(tc.tile_pool(name="crf", bufs=1))
        em_full = crf.tile([T, SB], F32, tag="emfull")
        eem = crf.tile([T, SB], F32, tag="eem")
        e0b = crf.tile([T, SB], BF16, tag="e0b")
        e1b = crf.tile([T, SB], BF16, tag="e1b")
        nc.sync.dma_start(e0b[:], emdbo[0])
        nc.sync.dma_start(
            e1b[:], emdbo[1].rearrange("j (t c) -> j t c", t=S)[:, ::-1, :])
        nc.vector.tensor_add(em_full[:], e0b[:], e1b[:])

        if stop_after == 'C':
            with tc.tile_pool(name="bail", bufs=1) as bp:
                bt = bp.tile([1, 1], F32)
                nc.vector.tensor_copy(bt[:], em_full[0:1, 0:1])
                nc.sync.dma_start(loss[:], bt[:])
            nc.compile()
            return nc

        # ---------------- Phase D: CRF ----------------
        with ExitStack() as ph:
            sp = ph.enter_context(tc.tile_pool(name="crftmp", bufs=2))
            big = ph.enter_context(tc.tile_pool(name="crfbig", bufs=2))
            ap_ = ph.enter_context(tc.tile_pool(name="alphas", bufs=2))

            cst = sp.tile([T, T], F32, tag="cst")        # transitions
            nc.sync.dma_start(cst[:], transm[:])
            st_sb = sp.tile([T, 1], F32, tag="stv")
            nc.sync.dma_start(st_sb[:], startv[:])
            en_sb = sp.tile([T, 1], F32, tag="env")
            nc.sync.dma_start(en_sb[:], endv[:])

            # --- numerator ---
            lab9 = big.tile([T, SB], I32, tag="big")
            nc.sync.dma_start(
                lab9[:],
                labT[:].rearrange("s c -> (s c)")[None, :].broadcast_to((T, SB)))
            labf = big.tile([T, SB], F32, tag="big")
            nc.vector.tensor_copy(labf[:], lab9[:])
            io9 = sp.tile([T, 1], I32, tag="io9")
            nc.gpsimd.iota(io9[:], pattern=[[0, 1]], base=0, channel_multiplier=1)
            io9f = sp.tile([T, 1], F32, tag="io9f")
            nc.vector.tensor_copy(io9f[:], io9[:])
            onehot = big.tile([T, SB], F32, tag="big")
            nc.vector.tensor_scalar(onehot[:], labf[:], io9f[:], None,
                                    op0=ALU.is_equal)
            gmul = big.tile([T, SB], F32, tag="big")
            nc.vector.tensor_mul(gmul[:], onehot[:], em_full[:])
            acc = sp.tile([T, b], F32, tag="acc")
            nc.vector.tensor_reduce(
                acc[:], gmul[:].rearrange("j (t c) -> j c t", c=b),
                op=ALU.add, axis=AXL.X)
            # start/end gold scores
            stsc = sp.tile([T, b], F32, tag="stsc")
            nc.vector.tensor_scalar_mul(stsc[:], onehot[:, 0:b], st_sb[:])
            nc.vector.tensor_add(acc[:], acc[:], stsc[:])
            ensc = sp.tile([T, b], F32, tag="ensc")
            nc.vector.tensor_scalar_mul(ensc[:], onehot[:, (S - 1) * b:S * b],
                                        en_sb[:])
            nc.vector.tensor_add(acc[:], acc[:], ensc[:])
            # transition gold scores: TH = T^T @ onehot ; V = TH * onehot_next
            numps = ExitStack()
            pp = numps.enter_context(
                tc.tile_pool(name="numps", bufs=2, space="PSUM"))
            for tc0 in range(0, S - 1, 32):
                tn = min(32, S - 1 - tc0)
                thp = pp.tile([T, 32 * b], F32, tag="thp")
                nc.tensor.matmul(thp[:, 0:tn * b], cst[:],
                                 onehot[:, tc0 * b:(tc0 + tn) * b],
                                 start=True, stop=True)
                v = sp.tile([T, 32 * b], F32, tag="v")
                nc.vector.tensor_mul(v[:, 0:tn * b], thp[:, 0:tn * b],
                                     onehot[:, (tc0 + 1) * b:(tc0 + 1 + tn) * b])
                vr = sp.tile([T, b], F32, tag="vr")
                nc.vector.tensor_reduce(
                    vr[:], v[:, 0:tn * b].rearrange("j (t c) -> j c t", c=b),
                    op=ALU.add, axis=AXL.X)
                nc.vector.tensor_add(acc[:], acc[:], vr[:])
            ones9 = sp.tile([T, 1], F32, tag="ones9")
            nc.vector.memset(ones9[:], 1.0)
            ones19 = sp.tile([1, T], F32, tag="ones19")
            nc.vector.memset(ones19[:], 1.0)
            nump = pp.tile([1, b], F32, tag="nump")
            nc.tensor.matmul(nump[:], ones9[:], acc[:], start=True, stop=True)
            num_sb = sp.tile([1, b], F32, tag="num")
            nc.vector.tensor_copy(num_sb[:], nump[:])
            numps.close()
            pp = ph.enter_context(tc.tile_pool(name="scanps", bufs=2, space="PSUM"))
            pp2 = ph.enter_context(tc.tile_pool(name="scanps2", bufs=2, space="PSUM"))

            # --- partition function (probability-domain scan, prescaled) ---
            NG = 2                     # interleaved batch groups
            gb = b // NG               # 8 sequences per group
            Em = sp.tile([T, T], F32, tag="Em")
            nc.scalar.activation(Em[:], cst[:], AFT.Exp)
            kneg = sp.tile([T, 1], F32, tag="kneg")
            nc.vector.memset(kneg[:], -KAPPA)
            nc.scalar.activation(eem[:], em_full[:], AFT.Exp, bias=kneg[:])
            es = sp.tile([T, 1], F32, tag="es")
            nc.scalar.activation(es[:], st_sb[:], AFT.Exp)
            ee = sp.tile([T, 1], F32, tag="ee")
            nc.scalar.activation(ee[:], en_sb[:], AFT.Exp)
            logacc = sp.tile([1, b], F32, tag="logacc")
            nc.vector.memset(logacc[:], 0.0)
            alphas = []
            for g in range(NG):
                al = ap_.tile([T, gb], F32, tag=f"al{g}")
                nc.vector.tensor_scalar_mul(al[:], eem[:, g * gb:(g + 1) * gb],
                                            es[:])
                alphas.append(al)
            for t in range(1, S):
                apss = []
                for g in range(NG):
                    aps = pp.tile([T, gb], F32, tag=f"aps{g}")
                    nc.tensor.matmul(aps[:], Em[:], alphas[g][:],
                                     start=True, stop=True)
                    apss.append(aps)
                for g in range(NG):
                    al = ap_.tile([T, gb], F32, tag=f"al{g}")
                    nc.vector.tensor_mul(
                        al[:], apss[g][:],
                        eem[:, t * b + g * gb:t * b + (g + 1) * gb])
                    alphas[g] = al
                if t % R == 0:
                    for g in range(NG):
                        ssum = pp2.tile([1, gb], F32, tag="ssum")
                        nc.tensor.matmul(ssum[:], ones9[:], alphas[g][:],
                                         start=True, stop=True)
                        ls = sp.tile([1, gb], F32, tag=f"ls{g}")
                        nc.scalar.activation(ls[:], ssum[:], AFT.Ln)
                        nc.vector.tensor_add(
                            logacc[:, g * gb:(g + 1) * gb],
                            logacc[:, g * gb:(g + 1) * gb], ls[:])
                        rc = sp.tile([1, gb], F32, tag=f"rc{g}")
                        nc.vector.reciprocal(rc[:], ssum[:])
                        bc = pp2.tile([T, gb], F32, tag="bc")
                        nc.tensor.matmul(bc[:], ones19[:], rc[:],
                                         start=True, stop=True)
                        al = ap_.tile([T, gb], F32, tag=f"al{g}")
                        nc.vector.tensor_mul(al[:], alphas[g][:], bc[:])
                        alphas[g] = al
            lv = sp.tile([1, b], F32, tag="lv")
            for g in range(NG):
                zp = pp.tile([1, gb], F32, tag=f"aps{g}")
                nc.tensor.matmul(zp[:], ee[:], alphas[g][:],
                                 start=True, stop=True)
                lz = sp.tile([1, gb], F32, tag=f"lz{g}")
                nc.scalar.activation(lz[:], zp[:], AFT.Ln)
                logz = sp.tile([1, gb], F32, tag=f"logz{g}")
                nc.vector.tensor_add(logz[:], lz[:],
                                     logacc[:, g * gb:(g + 1) * gb])
                # num - (logz + S*kappa)
                nc.vector.tensor_sub(lv[:, g * gb:(g + 1) * gb],
                                     num_sb[:, g * gb:(g + 1) * gb], logz[:])
            lvk = sp.tile([1, b], F32, tag="lvk")
            nc.vector.tensor_scalar_add(lvk[:], lv[:], -float(S) * KAPPA)
            tot = sp.tile([1, 1], F32, tag="tot")
            nc.vector.tensor_reduce(tot[:], lvk[:], op=ALU.add, axis=AXL.X)
            sc = sp.tile([1, 1], F32, tag="sc")
            nc.vector.tensor_scalar_mul(sc[:], tot[:], -1.0 / (2.0 * B_full))
            nc.sync.dma_start(lossdb[:], sc[:])
            nc.gpsimd.collective_compute(
                "AllReduce", ALU.add,
                replica_groups=[list(range(NCORES))],
                ins=[lossdb.opt()], outs=[lossout.opt()])
            lf = sp.tile([1, 1], F32, tag="lf")
            nc.sync.dma_start(lf[:], lossout[:])
            nc.sync.dma_start(loss[:], lf[:])

    nc.compile()
    return nc


# ---------------------------------------------------------------------------
# host-side sharding
# ---------------------------------------------------------------------------

def _perm_figo(HD):
    # torch gate order i,f,g,o -> f,i,g,o
    return np.concatenate([
        np.arange(HD, 2 * HD), np.arange(0, HD),
        np.arange(2 * HD, 3 * HD), np.arange(3 * HD, 4 * HD)])


def shard_inputs(inputs, b, S, E, HD, T):
    KE, NH = E // 128, HD // 128
    perm = _perm_figo(HD)
    bf = ml_dtypes.bfloat16
    x = np.asarray(inputs["x"], np.float32)
    labels = np.asarray(inputs["labels"]).astype(np.int32)
    trans = np.asarray(inputs["transitions"], np.float32)
    startv = np.asarray(inputs["start_trans"], np.float32).reshape(T, 1)
    endv = np.asarray(inputs["end_trans"], np.float32).reshape(T, 1)
    Wtag = np.asarray(inputs["W_tag"], np.float32)
    btag = np.asarray(inputs["b_tag"], np.float32).reshape(T, 1)
    identm = np.eye(128, dtype=np.float32).astype(bf)

    per_dir = {}
    for d, sfx in enumerate(("f", "b")):
        Wih = np.asarray(inputs[f"W_ih_{sfx}"], np.float32)[perm]
        Whh = np.asarray(inputs[f"W_hh_{sfx}"], np.float32)[perm]
        bias = (np.asarray(inputs[f"b_ih_{sfx}"], np.float32)
                + np.asarray(inputs[f"b_hh_{sfx}"], np.float32))[perm]
        per_dir[d] = dict(
            wihT=np.ascontiguousarray(
                Wih.T.reshape(KE, 128, 4 * HD)).astype(bf),
            whhT=np.ascontiguousarray(
                Whh.T.reshape(NH, 128, 4 * HD)).astype(bf),
            bias4=np.ascontiguousarray(
                bias.reshape(4 * NH, 128).T).astype(np.float32),
            wtagT=np.ascontiguousarray(
                Wtag[:, d * HD:(d + 1) * HD].T.reshape(NH, 128, T)).astype(bf),
            tagb=btag if d == 0 else np.zeros_like(btag),
            m0=np.full((T, 1), 1.0 - d, np.float32),
            m1=np.full((T, 1), float(d), np.float32),
        )

    in_maps = []
    for c in range(NCORES):
        d = c // NPAIR                      # 0 fwd, 1 bwd
        g = c % NPAIR                       # batch group
        xs = x[g * b:(g + 1) * b]           # (b, S, E)
        if d == 1:
            xs = xs[:, ::-1, :]
        xTc = np.ascontiguousarray(xs.transpose(2, 1, 0).reshape(KE, 128, S * b)
                                   ).astype(bf)
        m = dict(per_dir[d])
        m["xT"] = xTc
        m["labT"] = np.ascontiguousarray(labels[g * b:(g + 1) * b].T)
        m["transm"] = trans
        m["startv"] = startv
        m["endv"] = endv
        m["ident"] = identm
        in_maps.append(m)
    return in_maps


# ---------------------------------------------------------------------------
# entry point
# ---------------------------------------------------------------------------

_B, _S, _E, _HD, _T = 64, 512, 1024, 512, 9
_cache = {}


def _get_program():
    if "nc" not in _cache:
        _cache["nc"] = build_program(_B // NPAIR, _S, _E, _HD, _T, _B)
    return _cache["nc"]


def kernel(**inputs) -> np.ndarray:
    from concourse.bass_utils import run_bass_kernel_spmd
    nc = _get_program()
    in_maps = shard_inputs(inputs, _B // NPAIR, _S, _E, _HD, _T)
    res = run_bass_kernel_spmd(nc, in_maps, list(range(NCORES)))
    out = np.asarray(res.results[0]["loss"], np.float32).reshape(())
    return out


# revision 19
# speedup vs baseline: 1.2500x; 1.0698x over previous
"""BiLSTM-CRF loss kernel for Trainium2 (8 NeuronCores, Bass/Tile).

Strategy (v2)
-------------
Cores 0-3 run the FORWARD LSTM direction, cores 4-7 the BACKWARD direction
(fed time-reversed x), each over 16 of the 64 sequences (data-parallel over
batch within each direction).  Pair (c, c+4) handles the same 16 sequences.

Per core, one fused chunked loop (CH timesteps per chunk):
  - GX chunk n+1 (x @ W_ih^T + bias) is computed into SBUF (bf16),
    interleaved with the recurrence steps of chunk n so the big matmuls
    fill tensor-engine bubbles left by the serial LSTM chain.
  - LSTM cell per step: gate order f,i,g,o; W_hh matmuls accumulate into
    PSUM, GX is added by identity-stationary matmuls per gate block so the
    activations read PSUM directly and start early (sigmoid(f,i) under the
    g/o matmuls).  h is written bf16 straight into an SBUF history buffer.
  - em partials per chunk from the SBUF h history; two masked slots are
    written to DRAM and pair-AllReduced (fwd slot / time-reversed bwd slot).
  - CRF: gold score via one-hot matmul reductions; partition function via
    probability-domain scan with a constant e^-kappa prescale folded into
    exp(em), two interleaved batch groups to hide semaphore latency, and a
    proper rescale only every R steps.  Final loss AllReduce over 8 cores.
"""

import sys

sys.path.insert(0, "/opt/trn_rl_repo")

import numpy as np
import ml_dtypes
from contextlib import ExitStack

import concourse.bass as bass
import concourse.bacc as bacc
import concourse.tile as tile
import concourse.mybir as mybir

F32 = mybir.dt.float32
BF16 = mybir.dt.bfloat16
I32 = mybir.dt.int32
AFT = mybir.ActivationFunctionType
ALU = mybir.AluOpType
AXL = mybir.AxisListType

NCORES = 8
NPAIR = 4  # fwd cores 0..3, bwd cores 4..7
KAPPA = 2.2  # CRF scan prescale: eem = exp(em - KAPPA)


# ---------------------------------------------------------------------------
# program builder (SPMD: one program, per-core divergence is data only)
# ---------------------------------------------------------------------------

def build_program(b, S, E, HD, T, B_full, CH=32, R=64, stop_after=None):
    """b: sequences per core; returns the Bass program."""
    KE = E // 128          # input-proj K tiles
    NH = HD // 128         # hidden K tiles (= h tiles)
    NM = 4 * NH            # gate m-tiles (permuted order f,i,g,o)
    SB = S * b             # (t, b) flattened column count
    W = NH * b             # per-step h column width  (64)
    SBc = CH * b           # columns per chunk         (512)
    NCHK = S // CH
    assert S % CH == 0 and CH % 2 == 0 and NM == 16 and CH >= 2 * NM // 2

    nc = bacc.Bacc("TRN2", target_bir_lowering=False, debug=False,
                   num_devices=NCORES)

    # ---- I/O ----
    xT = nc.dram_tensor("xT", [KE, 128, SB], BF16, kind="ExternalInput")
    wihT = nc.dram_tensor("wihT", [KE, 128, 4 * HD], BF16, kind="ExternalInput")
    whhT = nc.dram_tensor("whhT", [NH, 128, 4 * HD], BF16, kind="ExternalInput")
    bias4 = nc.dram_tensor("bias4", [128, NM], F32, kind="ExternalInput")
    ident = nc.dram_tensor("ident", [128, 128], BF16, kind="ExternalInput")
    wtagT = nc.dram_tensor("wtagT", [NH, 128, T], BF16, kind="ExternalInput")
    tagb = nc.dram_tensor("tagb", [T, 1], F32, kind="ExternalInput")
    m0 = nc.dram_tensor("m0", [T, 1], F32, kind="ExternalInput")
    m1 = nc.dram_tensor("m1", [T, 1], F32, kind="ExternalInput")
    labT = nc.dram_tensor("labT", [S, b], I32, kind="ExternalInput")
    transm = nc.dram_tensor("transm", [T, T], F32, kind="ExternalInput")
    startv = nc.dram_tensor("startv", [T, 1], F32, kind="ExternalInput")
    endv = nc.dram_tensor("endv", [T, 1], F32, kind="ExternalInput")
    loss = nc.dram_tensor("loss", [1, 1], F32, kind="ExternalOutput")

    with tile.TileContext(nc) as tc, ExitStack() as top:
        dram = top.enter_context(tc.tile_pool(name="dram", bufs=1, space="DRAM"))
        emdbA = dram.tile([2, T, SB // 2], BF16)
        emdbB = dram.tile([2, T, SB // 2], BF16)
        emdboA = dram.tile([2, T, SB // 2], BF16)
        emdboB = dram.tile([2, T, SB // 2], BF16)
        lossdb = dram.tile([1, 1], F32)
        lossout = dram.tile([1, 1], F32)

        # ============== fused phase A+B+C (chunked) ==============
        ab = ExitStack()
        persist = ab.enter_context(tc.tile_pool(name="persist", bufs=1))
        hist = persist.tile([128, S * W], BF16)      # h history [t, k, b]
        c_sb = persist.tile([128, W], F32)
        wp = ab.enter_context(tc.tile_pool(name="weights", bufs=1))
        wih_sb = wp.tile([128, KE * 4 * HD], BF16)
        whh_sb = wp.tile([128, NH * 4 * HD], BF16)
        bias_sb = wp.tile([128, NM], F32)
        ident_sb = wp.tile([128, 128], BF16)
        wtag_sb = wp.tile([128, NH * T], BF16)
        tagb_sb = wp.tile([T, 1], F32)
        m0_sb = wp.tile([T, 1], F32)
        m1_sb = wp.tile([T, 1], F32)
        nc.sync.dma_start(wih_sb[:], wihT[:])
        nc.sync.dma_start(whh_sb[:], whhT[:])
        nc.sync.dma_start(bias_sb[:], bias4[:])
        nc.sync.dma_start(ident_sb[:], ident[:])
        nc.sync.dma_start(wtag_sb[:], wtagT[:])
        nc.sync.dma_start(tagb_sb[:], tagb[:])
        nc.sync.dma_start(m0_sb[:], m0[:])
        nc.sync.dma_start(m1_sb[:], m1[:])

        xp = ab.enter_context(tc.tile_pool(name="xin", bufs=2))
        gxsp = ab.enter_context(tc.tile_pool(name="gxs", bufs=2))
        gxps = ab.enter_context(tc.tile_pool(name="gxps", bufs=1, space="PSUM"))
        rp = ab.enter_context(tc.tile_pool(name="recps", bufs=2, space="PSUM"))
        ep = ab.enter_context(tc.tile_pool(name="emps", bufs=1, space="PSUM"))
        tp = ab.enter_context(tc.tile_pool(name="steptmp", bufs=2))
        sp2 = ab.enter_context(tc.tile_pool(name="emtmp", bufs=2))

        def emit_gx_mtile(m, xt_sb, gxc):
            # gxc layout: [128, (tt, m, b)] — per-step gx blocks contiguous
            ps = gxps.tile([128, SBc], F32)
            for k in range(KE):
                nc.tensor.matmul(
                    ps[:],
                    wih_sb[:, k * 4 * HD + m * 128:k * 4 * HD + (m + 1) * 128],
                    xt_sb[:, k * SBc:(k + 1) * SBc],
                    start=(k == 0), stop=(k == KE - 1))
            out_ap = gxc[:].rearrange("p (t m c) -> p m t c", m=NM, c=b)[:, m]
            nc.vector.tensor_scalar(out_ap, ps[:].rearrange(
                "p (t c) -> p t c", c=b), bias_sb[:, m:m + 1], None, op0=ALU.add)

        # prologue: x + GX for chunk 0
        xt_sb = xp.tile([128, KE * SBc], BF16)
        nc.sync.dma_start(xt_sb[:], xT[:, :, 0:SBc])
        gxc = gxsp.tile([128, NM * SBc], BF16)
        for m in range(NM):
            emit_gx_mtile(m, xt_sb, gxc)

        for n in range(NCHK):
            gx_cur = gxc
            if n + 1 < NCHK:
                xt_sb = xp.tile([128, KE * SBc], BF16)
                nc.sync.dma_start(
                    xt_sb[:], xT[:, :, (n + 1) * SBc:(n + 2) * SBc])
                gxc = gxsp.tile([128, NM * SBc], BF16)

            for tt in range(CH):
                t = n * CH + tt

                def gx_ap(mlo, mn):
                    # contiguous [128, mn*b] slice of step tt's gx block
                    return gx_cur[:, tt * NM * b + mlo * b:
                                  tt * NM * b + (mlo + mn) * b]

                if t == 0:
                    sig = tp.tile([128, 3 * W], F32, tag="sig")
                    nc.scalar.activation(sig[:, 0:2 * W], gx_ap(0, 2 * NH),
                                         AFT.Sigmoid)
                    tg = tp.tile([128, W], F32, tag="tg")
                    nc.scalar.activation(tg[:], gx_ap(2 * NH, NH), AFT.Tanh)
                    nc.scalar.activation(sig[:, 2 * W:3 * W], gx_ap(3 * NH, NH),
                                         AFT.Sigmoid)
                    nc.vector.tensor_mul(c_sb[:], sig[:, W:2 * W], tg[:])
                else:
                    h_prev = hist[:, (t - 1) * W:t * W]

                    # own PSUM bank per gate block: start=True zeroing is
                    # bank-granular, so blocks must not share banks with
                    # regions still being read by the activations.
                    # The identity gx-adds go FIRST (start=True) — no h
                    # dependency, so they run during the previous step's
                    # activation tail while the PE is otherwise idle.
                    psfi = rp.tile([128, 2 * W], F32, tag="psfi")
                    psg = rp.tile([128, W], F32, tag="psg")
                    pso = rp.tile([128, W], F32, tag="pso")
                    for pst, mlo, mn in ((psfi, 0, 2 * NH), (psg, 2 * NH, NH),
                                         (pso, 3 * NH, NH)):
                        nc.tensor.matmul(
                            pst[:, 0:mn * b], ident_sb[:], gx_ap(mlo, mn),
                            start=True, stop=False, skip_group_check=True)

                    def whh_block(pst, mlo, mn):
                        for mi in range(mn):
                            mm = mlo + mi
                            for kt in range(NH):
                                nc.tensor.matmul(
                                    pst[:, mi * b:(mi + 1) * b],
                                    whh_sb[:, kt * 4 * HD + mm * 128:
                                           kt * 4 * HD + (mm + 1) * 128],
                                    h_prev[:, kt * b:(kt + 1) * b],
                                    start=False, stop=(kt == NH - 1),
                                    skip_group_check=True)

                    whh_block(psfi, 0, 2 * NH)    # f, i
                    sig = tp.tile([128, 3 * W], F32, tag="sig")
                    nc.scalar.activation(sig[:, 0:2 * W], psfi[:], AFT.Sigmoid)
                    whh_block(psg, 2 * NH, NH)    # g
                    tg = tp.tile([128, W], F32, tag="tg")
                    nc.scalar.activation(tg[:], psg[:], AFT.Tanh)
                    whh_block(pso, 3 * NH, NH)    # o
                    t1 = tp.tile([128, W], F32, tag="t1")
                    nc.vector.tensor_mul(t1[:], sig[:, 0:W], c_sb[:])
                    t2 = tp.tile([128, W], F32, tag="t2")
                    nc.vector.tensor_mul(t2[:], sig[:, W:2 * W], tg[:])
                    nc.vector.tensor_add(c_sb[:], t1[:], t2[:])
                    nc.scalar.activation(sig[:, 2 * W:3 * W], pso[:],
                                         AFT.Sigmoid)
                tanc = tp.tile([128, W], F32, tag="tanc")
                nc.scalar.activation(tanc[:], c_sb[:], AFT.Tanh)
                nc.vector.tensor_mul(hist[:, t * W:(t + 1) * W],
                                     sig[:, 2 * W:3 * W], tanc[:])

                # interleave GX production for chunk n+1 into this chunk
                if n + 1 < NCHK and tt % 2 == 1 and tt // 2 < NM:
                    emit_gx_mtile(tt // 2, xt_sb, gxc)

            # ---- em partial for chunk n ----
            hv = hist[:, n * CH * W:(n + 1) * CH * W].rearrange(
                "p (t k c) -> p t k c", t=CH, k=NH)
            pse = ep.tile([T, SBc], F32)
            for kt in range(NH):
                nc.tensor.matmul(
                    pse[:].rearrange("p (t c) -> p t c", t=CH),
                    wtag_sb[:, kt * T:(kt + 1) * T],
                    hv[:, :, kt, :],
                    start=(kt == 0), stop=(kt == NH - 1))
            emdb_h, noff = (emdbA, n) if n < NCHK // 2 else (emdbB, n - NCHK // 2)
            s0 = sp2.tile([T, SBc], BF16, tag="s0")
            nc.vector.tensor_scalar(s0[:], pse[:], tagb_sb[:], m0_sb[:],
                                    op0=ALU.add, op1=ALU.mult)
            nc.sync.dma_start(emdb_h[0, :, noff * SBc:(noff + 1) * SBc], s0[:])
            s1 = sp2.tile([T, SBc], BF16, tag="s1")
            nc.vector.tensor_scalar(s1[:], pse[:], tagb_sb[:], m1_sb[:],
                                    op0=ALU.add, op1=ALU.mult)
            nc.sync.dma_start(emdb_h[1, :, noff * SBc:(noff + 1) * SBc], s1[:])
            if n == NCHK // 2 - 1:
                # first-half em AllReduce overlaps the second half of the
                # recurrence
                nc.gpsimd.collective_compute(
                    "AllReduce", ALU.add,
                    replica_groups=[[c, c + NPAIR] for c in range(NPAIR)],
                    ins=[emdbA.opt()], outs=[emdboA.opt()])

        if stop_after == 'B':
            with tc.tile_pool(name="bail", bufs=1) as bp:
                bt = bp.tile([1, 1], F32)
                nc.vector.tensor_copy(bt[:], c_sb[0:1, 0:1])
                nc.sync.dma_start(loss[:], bt[:])
            ab.close()
            nc.compile()
            return nc

        ab.close()

        # ---- second-half pair AllReduce of em slots ----
        nc.gpsimd.collective_compute(
            "AllReduce", ALU.add,
            replica_groups=[[c, c + NPAIR] for c in range(NPAIR)],
            ins=[emdbB.opt()], outs=[emdboB.opt()])

        crf = top.enter_context(tc.tile_pool(name="crf", bufs=1))
        em_full = crf.tile([T, SB], F32, tag="emfull")
        eem = crf.tile([T, SB], F32, tag="eem")
        e0b = crf.tile([T, SB], BF16, tag="e0b")
        e1b = crf.tile([T, SB], BF16, tag="e1b")
        nc.sync.dma_start(e0b[:, 0:SB // 2], emdboA[0])
        nc.sync.dma_start(e0b[:, SB // 2:SB], emdboB[0])
        # bwd slot is time-reversed: storage half A (t'=0..S/2-1) covers real
        # t = S-1..S/2, i.e. the SECOND half of e1b, reversed (and vice versa)
        nc.sync.dma_start(
            e1b[:, (S // 2) * b:S * b],
            emdboA[1].rearrange("j (t c) -> j t c", t=S // 2)[:, ::-1, :])
        nc.sync.dma_start(
            e1b[:, 0:(S // 2) * b],
            emdboB[1].rearrange("j (t c) -> j t c", t=S // 2)[:, ::-1, :])
        nc.vector.tensor_add(em_full[:], e0b[:], e1b[:])

        if stop_after == 'C':
            with tc.tile_pool(name="bail", bufs=1) as bp:
                bt = bp.tile([1, 1], F32)
                nc.vector.tensor_copy(bt[:], em_full[0:1, 0:1])
                nc.sync.dma_start(loss[:], bt[:])
            nc.compile()
            return nc

        # ---------------- Phase D: CRF ----------------
        with ExitStack() as ph:
            sp = ph.enter_context(tc.tile_pool(name="crftmp", bufs=2))
            big = ph.enter_context(tc.tile_pool(name="crfbig", bufs=2))
            ap_ = ph.enter_context(tc.tile_pool(name="alphas", bufs=2))

            cst = sp.tile([T, T], F32, tag="cst")        # transitions
            nc.sync.dma_start(cst[:], transm[:])
            st_sb = sp.tile([T, 1], F32, tag="stv")
            nc.sync.dma_start(st_sb[:], startv[:])
            en_sb = sp.tile([T, 1], F32, tag="env")
            nc.sync.dma_start(en_sb[:], endv[:])

            # --- numerator ---
            lab9 = big.tile([T, SB], I32, tag="big")
            nc.sync.dma_start(
                lab9[:],
                labT[:].rearrange("s c -> (s c)")[None, :].broadcast_to((T, SB)))
            labf = big.tile([T, SB], F32, tag="big")
            nc.vector.tensor_copy(labf[:], lab9[:])
            io9 = sp.tile([T, 1], I32, tag="io9")
            nc.gpsimd.iota(io9[:], pattern=[[0, 1]], base=0, channel_multiplier=1)
            io9f = sp.tile([T, 1], F32, tag="io9f")
            nc.vector.tensor_copy(io9f[:], io9[:])
            onehot = big.tile([T, SB], F32, tag="big")
            nc.vector.tensor_scalar(onehot[:], labf[:], io9f[:], None,
                                    op0=ALU.is_equal)
            gmul = big.tile([T, SB], F32, tag="big")
            nc.vector.tensor_mul(gmul[:], onehot[:], em_full[:])
            acc = sp.tile([T, b], F32, tag="acc")
            nc.vector.tensor_reduce(
                acc[:], gmul[:].rearrange("j (t c) -> j c t", c=b),
                op=ALU.add, axis=AXL.X)
            # start/end gold scores
            stsc = sp.tile([T, b], F32, tag="stsc")
            nc.vector.tensor_scalar_mul(stsc[:], onehot[:, 0:b], st_sb[:])
            nc.vector.tensor_add(acc[:], acc[:], stsc[:])
            ensc = sp.tile([T, b], F32, tag="ensc")
            nc.vector.tensor_scalar_mul(ensc[:], onehot[:, (S - 1) * b:S * b],
                                        en_sb[:])
            nc.vector.tensor_add(acc[:], acc[:], ensc[:])
            # transition gold scores: TH = T^T @ onehot ; V = TH * onehot_next
            numps = ExitStack()
            pp = numps.enter_context(
                tc.tile_pool(name="numps", bufs=2, space="PSUM"))
            for tc0 in range(0, S - 1, 32):
                tn = min(32, S - 1 - tc0)
                thp = pp.tile([T, 32 * b], F32, tag="thp")
                nc.tensor.matmul(thp[:, 0:tn * b], cst[:],
                                 onehot[:, tc0 * b:(tc0 + tn) * b],
                                 start=True, stop=True)
                v = sp.tile([T, 32 * b], F32, tag="v")
                nc.vector.tensor_mul(v[:, 0:tn * b], thp[:, 0:tn * b],
                                     onehot[:, (tc0 + 1) * b:(tc0 + 1 + tn) * b])
                vr = sp.tile([T, b], F32, tag="vr")
                nc.vector.tensor_reduce(
                    vr[:], v[:, 0:tn * b].rearrange("j (t c) -> j c t", c=b),
                    op=ALU.add, axis=AXL.X)
                nc.vector.tensor_add(acc[:], acc[:], vr[:])
            ones9 = sp.tile([T, 1], F32, tag="ones9")
            nc.vector.memset(ones9[:], 1.0)
            ones19 = sp.tile([1, T], F32, tag="ones19")
            nc.vector.memset(ones19[:], 1.0)
            nump = pp.tile([1, b], F32, tag="nump")
            nc.tensor.matmul(nump[:], ones9[:], acc[:], start=True, stop=True)
            num_sb = sp.tile([1, b], F32, tag="num")
            nc.vector.tensor_copy(num_sb[:], nump[:])
            numps.close()
            pp = ph.enter_context(tc.tile_pool(name="scanps", bufs=2, space="PSUM"))
            pp2 = ph.enter_context(tc.tile_pool(name="scanps2", bufs=2, space="PSUM"))

            # --- partition function (probability-domain scan, prescaled) ---
            NG = 2                     # interleaved batch groups
            gb = b // NG               # 8 sequences per group
            Em = sp.tile([T, T], F32, tag="Em")
            nc.scalar.activation(Em[:], cst[:], AFT.Exp)
            kneg = sp.tile([T, 1], F32, tag="kneg")
            nc.vector.memset(kneg[:], -KAPPA)
            nc.scalar.activation(eem[:], em_full[:], AFT.Exp, bias=kneg[:])
            es = sp.tile([T, 1], F32, tag="es")
            nc.scalar.activation(es[:], st_sb[:], AFT.Exp)
            ee = sp.tile([T, 1], F32, tag="ee")
            nc.scalar.activation(ee[:], en_sb[:], AFT.Exp)
            logacc = sp.tile([1, b], F32, tag="logacc")
            nc.vector.memset(logacc[:], 0.0)
            alphas = []
            for g in range(NG):
                al = ap_.tile([T, gb], F32, tag=f"al{g}")
                nc.vector.tensor_scalar_mul(al[:], eem[:, g * gb:(g + 1) * gb],
                                            es[:])
                alphas.append(al)
            for t in range(1, S):
                apss = []
                for g in range(NG):
                    aps = pp.tile([T, gb], F32, tag=f"aps{g}")
                    nc.tensor.matmul(aps[:], Em[:], alphas[g][:],
                                     start=True, stop=True)
                    apss.append(aps)
                for g in range(NG):
                    al = ap_.tile([T, gb], F32, tag=f"al{g}")
                    nc.vector.tensor_mul(
                        al[:], apss[g][:],
                        eem[:, t * b + g * gb:t * b + (g + 1) * gb])
                    alphas[g] = al
                if t % R == 0:
                    for g in range(NG):
                        ssum = pp2.tile([1, gb], F32, tag="ssum")
                        nc.tensor.matmul(ssum[:], ones9[:], alphas[g][:],
                                         start=True, stop=True)
                        ls = sp.tile([1, gb], F32, tag=f"ls{g}")
                        nc.scalar.activation(ls[:], ssum[:], AFT.Ln)
                        nc.vector.tensor_add(
                            logacc[:, g * gb:(g + 1) * gb],
                            logacc[:, g * gb:(g + 1) * gb], ls[:])
                        rc = sp.tile([1, gb], F32, tag=f"rc{g}")
                        nc.vector.reciprocal(rc[:], ssum[:])
                        bc = pp2.tile([T, gb], F32, tag="bc")
                        nc.tensor.matmul(bc[:], ones19[:], rc[:],
                                         start=True, stop=True)
                        al = ap_.tile([T, gb], F32, tag=f"al{g}")
                        nc.vector.tensor_mul(al[:], alphas[g][:], bc[:])
                        alphas[g] = al
            lv = sp.tile([1, b], F32, tag="lv")
            for g in range(NG):
                zp = pp.tile([1, gb], F32, tag=f"aps{g}")
                nc.tensor.matmul(zp[:], ee[:], alphas[g][:],
                                 start=True, stop=True)
                lz = sp.tile([1, gb], F32, tag=f"lz{g}")
                nc.scalar.activation(lz[:], zp[:], AFT.Ln)
                logz = sp.tile([1, gb], F32, tag=f"logz{g}")
                nc.vector.tensor_add(logz[:], lz[:],
                                     logacc[:, g * gb:(g + 1) * gb])
                # num - (logz + S*kappa)
                nc.vector.tensor_sub(lv[:, g * gb:(g + 1) * gb],
                                     num_sb[:, g * gb:(g + 1) * gb], logz[:])
            lvk = sp.tile([1, b], F32, tag="lvk")
            nc.vector.tensor_scalar_add(lvk[:], lv[:], -float(S) * KAPPA)
            tot = sp.tile([1, 1], F32, tag="tot")
            nc.vector.tensor_reduce(tot[:], lvk[:], op=ALU.add, axis=AXL.X)
            sc = sp.tile([1, 1], F32, tag="sc")
            nc.vector.tensor_scalar_mul(sc[:], tot[:], -1.0 / (2.0 * B_full))
            nc.sync.dma_start(lossdb[:], sc[:])
            nc.gpsimd.collective_compute(
                "AllReduce", ALU.add,
                replica_groups=[list(range(NCORES))],
                ins=[lossdb.opt()], outs=[lossout.opt()])
            lf = sp.tile([1, 1], F32, tag="lf")
            nc.sync.dma_start(lf[:], lossout[:])
            nc.sync.dma_start(loss[:], lf[:])

    nc.compile()
    return nc


# ---------------------------------------------------------------------------
# host-side sharding
# ---------------------------------------------------------------------------

def _perm_figo(HD):
    # torch gate order i,f,g,o -> f,i,g,o
    return np.concatenate([
        np.arange(HD, 2 * HD), np.arange(0, HD),
        np.arange(2 * HD, 3 * HD), np.arange(3 * HD, 4 * HD)])


def shard_inputs(inputs, b, S, E, HD, T):
    KE, NH = E // 128, HD // 128
    perm = _perm_figo(HD)
    bf = ml_dtypes.bfloat16
    x = np.asarray(inputs["x"], np.float32)
    labels = np.asarray(inputs["labels"]).astype(np.int32)
    trans = np.asarray(inputs["transitions"], np.float32)
    startv = np.asarray(inputs["start_trans"], np.float32).reshape(T, 1)
    endv = np.asarray(inputs["end_trans"], np.float32).reshape(T, 1)
    Wtag = np.asarray(inputs["W_tag"], np.float32)
    btag = np.asarray(inputs["b_tag"], np.float32).reshape(T, 1)
    identm = np.eye(128, dtype=np.float32).astype(bf)

    per_dir = {}
    for d, sfx in enumerate(("f", "b")):
        Wih = np.asarray(inputs[f"W_ih_{sfx}"], np.float32)[perm]
        Whh = np.asarray(inputs[f"W_hh_{sfx}"], np.float32)[perm]
        bias = (np.asarray(inputs[f"b_ih_{sfx}"], np.float32)
                + np.asarray(inputs[f"b_hh_{sfx}"], np.float32))[perm]
        per_dir[d] = dict(
            wihT=np.ascontiguousarray(
                Wih.T.reshape(KE, 128, 4 * HD)).astype(bf),
            whhT=np.ascontiguousarray(
                Whh.T.reshape(NH, 128, 4 * HD)).astype(bf),
            bias4=np.ascontiguousarray(
                bias.reshape(4 * NH, 128).T).astype(np.float32),
            wtagT=np.ascontiguousarray(
                Wtag[:, d * HD:(d + 1) * HD].T.reshape(NH, 128, T)).astype(bf),
            tagb=btag if d == 0 else np.zeros_like(btag),
            m0=np.full((T, 1), 1.0 - d, np.float32),
            m1=np.full((T, 1), float(d), np.float32),
        )

    in_maps = []
    for c in range(NCORES):
        d = c // NPAIR                      # 0 fwd, 1 bwd
        g = c % NPAIR                       # batch group
        xs = x[g * b:(g + 1) * b]           # (b, S, E)
        if d == 1:
            xs = xs[:, ::-1, :]
        xTc = np.ascontiguousarray(xs.transpose(2, 1, 0).reshape(KE, 128, S * b)
                                   ).astype(bf)
        m = dict(per_dir[d])
        m["xT"] = xTc
        m["labT"] = np.ascontiguousarray(labels[g * b:(g + 1) * b].T)
        m["transm"] = trans
        m["startv"] = startv
        m["endv"] = endv
        m["ident"] = identm
        in_maps.append(m)
    return in_maps


# ---------------------------------------------------------------------------
# entry point
# ---------------------------------------------------------------------------

_B, _S, _E, _HD, _T = 64, 512, 1024, 512, 9
_cache = {}


def _get_program():
    if "nc" not in _cache:
        _cache["nc"] = build_program(_B // NPAIR, _S, _E, _HD, _T, _B)
    return _cache["nc"]


def kernel(**inputs) -> np.ndarray:
    from concourse.bass_utils import run_bass_kernel_spmd
    nc = _get_program()
    in_maps = shard_inputs(inputs, _B // NPAIR, _S, _E, _HD, _T)
    res = run_bass_kernel_spmd(nc, in_maps, list(range(NCORES)))
    out = np.asarray(res.results[0]["loss"], np.float32).reshape(())
    return out


# revision 27
# speedup vs baseline: 1.2628x; 1.0102x over previous
"""BiLSTM-CRF loss kernel for Trainium2 (8 NeuronCores, Bass/Tile).

Strategy (v2)
-------------
Cores 0-3 run the FORWARD LSTM direction, cores 4-7 the BACKWARD direction
(fed time-reversed x), each over 16 of the 64 sequences (data-parallel over
batch within each direction).  Pair (c, c+4) handles the same 16 sequences.

Per core, one fused chunked loop (CH timesteps per chunk):
  - GX chunk n+1 (x @ W_ih^T + bias) is computed into SBUF (bf16),
    interleaved with the recurrence steps of chunk n so the big matmuls
    fill tensor-engine bubbles left by the serial LSTM chain.
  - LSTM cell per step: gate order f,i,g,o; W_hh matmuls accumulate into
    PSUM, GX is added by identity-stationary matmuls per gate block so the
    activations read PSUM directly and start early (sigmoid(f,i) under the
    g/o matmuls).  h is written bf16 straight into an SBUF history buffer.
  - em partials per chunk from the SBUF h history; two masked slots are
    written to DRAM and pair-AllReduced (fwd slot / time-reversed bwd slot).
  - CRF: gold score via one-hot matmul reductions; partition function via
    probability-domain scan with a constant e^-kappa prescale folded into
    exp(em), two interleaved batch groups to hide semaphore latency, and a
    proper rescale only every R steps.  Final loss AllReduce over 8 cores.
"""

import sys

sys.path.insert(0, "/opt/trn_rl_repo")

import numpy as np
import ml_dtypes
from contextlib import ExitStack

import concourse.bass as bass
import concourse.bacc as bacc
import concourse.tile as tile
import concourse.mybir as mybir

F32 = mybir.dt.float32
BF16 = mybir.dt.bfloat16
I32 = mybir.dt.int32
AFT = mybir.ActivationFunctionType
ALU = mybir.AluOpType
AXL = mybir.AxisListType

NCORES = 8
NPAIR = 4  # fwd cores 0..3, bwd cores 4..7
KAPPA = 2.2  # CRF scan prescale: eem = exp(em - KAPPA)


# ---------------------------------------------------------------------------
# program builder (SPMD: one program, per-core divergence is data only)
# ---------------------------------------------------------------------------

def build_program(b, S, E, HD, T, B_full, CH=32, R=64, stop_after=None):
    """b: sequences per core; returns the Bass program."""
    KE = E // 128          # input-proj K tiles
    NH = HD // 128         # hidden K tiles (= h tiles)
    NM = 4 * NH            # gate m-tiles (permuted order f,i,g,o)
    SB = S * b             # (t, b) flattened column count
    W = NH * b             # per-step h column width  (64)
    SBc = CH * b           # columns per chunk         (512)
    NCHK = S // CH
    assert S % CH == 0 and CH % 2 == 0 and NM == 16 and CH >= 2 * NM // 2

    nc = bacc.Bacc("TRN2", target_bir_lowering=False, debug=False,
                   num_devices=NCORES)

    # ---- I/O ----
    xT = nc.dram_tensor("xT", [KE, 128, SB], BF16, kind="ExternalInput")
    wihT = nc.dram_tensor("wihT", [KE, 128, 4 * HD], BF16, kind="ExternalInput")
    whhT = nc.dram_tensor("whhT", [NH, 128, 4 * HD], BF16, kind="ExternalInput")
    bias4 = nc.dram_tensor("bias4", [128, NM], F32, kind="ExternalInput")
    ident = nc.dram_tensor("ident", [128, 128], BF16, kind="ExternalInput")
    wtagT = nc.dram_tensor("wtagT", [NH, 128, T], BF16, kind="ExternalInput")
    tagb = nc.dram_tensor("tagb", [T, 1], F32, kind="ExternalInput")
    m0 = nc.dram_tensor("m0", [T, 1], F32, kind="ExternalInput")
    m1 = nc.dram_tensor("m1", [T, 1], F32, kind="ExternalInput")
    labT = nc.dram_tensor("labT", [S, b], I32, kind="ExternalInput")
    transm = nc.dram_tensor("transm", [T, T], F32, kind="ExternalInput")
    startv = nc.dram_tensor("startv", [T, 1], F32, kind="ExternalInput")
    endv = nc.dram_tensor("endv", [T, 1], F32, kind="ExternalInput")
    loss = nc.dram_tensor("loss", [1, 1], F32, kind="ExternalOutput")

    with tile.TileContext(nc) as tc, ExitStack() as top:
        dram = top.enter_context(tc.tile_pool(name="dram", bufs=1, space="DRAM"))
        # em exchange segments (chunk ranges): ARs for all but the last overlap
        # the remaining recurrence chunks
        NCHK0 = S // CH
        SEGS = [(0, NCHK0 // 2), (NCHK0 // 2, 3 * NCHK0 // 4),
                (3 * NCHK0 // 4, NCHK0)]
        emseg = []
        for si, (c0, c1) in enumerate(SEGS):
            w = (c1 - c0) * CH * b
            emin_t = dram.tile([2, T, w], BF16, tag=f"emin{si}")
            emout_t = dram.tile([2, T, w], BF16, tag=f"emout{si}")
            emseg.append((emin_t, emout_t, c0, c1))
        lossdb = dram.tile([1, 1], F32)
        lossout = dram.tile([1, 1], F32)

        # ============== fused phase A+B+C (chunked) ==============
        ab = ExitStack()
        persist = ab.enter_context(tc.tile_pool(name="persist", bufs=1))
        hist = persist.tile([128, S * W], BF16)      # h history [t, k, b]
        c_sb = persist.tile([128, W], F32)
        wp = ab.enter_context(tc.tile_pool(name="weights", bufs=1))
        wih_sb = wp.tile([128, KE * 4 * HD], BF16)
        whh_sb = wp.tile([128, NH * 4 * HD], BF16)
        bias_sb = wp.tile([128, NM], F32)
        ident_sb = wp.tile([128, 128], BF16)
        wtag_sb = wp.tile([128, NH * T], BF16)
        tagb_sb = wp.tile([T, 1], F32)
        m0_sb = wp.tile([T, 1], F32)
        m1_sb = wp.tile([T, 1], F32)
        nc.sync.dma_start(wih_sb[:], wihT[:])
        nc.sync.dma_start(whh_sb[:], whhT[:])
        nc.sync.dma_start(bias_sb[:], bias4[:])
        nc.sync.dma_start(ident_sb[:], ident[:])
        nc.sync.dma_start(wtag_sb[:], wtagT[:])
        nc.sync.dma_start(tagb_sb[:], tagb[:])
        nc.sync.dma_start(m0_sb[:], m0[:])
        nc.sync.dma_start(m1_sb[:], m1[:])

        xp = ab.enter_context(tc.tile_pool(name="xin", bufs=2))
        gxsp = ab.enter_context(tc.tile_pool(name="gxs", bufs=2))
        gxps = ab.enter_context(tc.tile_pool(name="gxps", bufs=1, space="PSUM"))
        rp = ab.enter_context(tc.tile_pool(name="recps", bufs=2, space="PSUM"))
        ep = ab.enter_context(tc.tile_pool(name="emps", bufs=1, space="PSUM"))
        tp = ab.enter_context(tc.tile_pool(name="steptmp", bufs=2))
        sp2 = ab.enter_context(tc.tile_pool(name="emtmp", bufs=2))

        gx_ps_live = [None]

        def emit_gx_half(m, half, xt_sb, gxc):
            # gxc layout: [128, (tt, m, b)] — per-step gx blocks contiguous.
            # One m-tile is spread over two consecutive steps (4 K-tiles each)
            # so every step's stall window gets PE filler.
            if half == 0:
                ps_new = gxps.tile([128, SBc], F32)
                gx_ps_live[0] = ps_new
            ps = gx_ps_live[0]
            for ki in range(KE // 2):
                k = half * (KE // 2) + ki
                nc.tensor.matmul(
                    ps[:],
                    wih_sb[:, k * 4 * HD + m * 128:k * 4 * HD + (m + 1) * 128],
                    xt_sb[:, k * SBc:(k + 1) * SBc],
                    start=(k == 0), stop=(k == KE - 1))
            if half == 1:
                out_ap = gxc[:].rearrange("p (t m c) -> p m t c",
                                          m=NM, c=b)[:, m]
                nc.vector.tensor_scalar(out_ap, ps[:].rearrange(
                    "p (t c) -> p t c", c=b), bias_sb[:, m:m + 1], None,
                    op0=ALU.add)

        # prologue: x + GX for chunk 0
        xt_sb = xp.tile([128, KE * SBc], BF16)
        nc.sync.dma_start(xt_sb[:], xT[:, :, 0:SBc])
        gxc = gxsp.tile([128, NM * SBc], BF16)
        for m in range(NM):
            emit_gx_half(m, 0, xt_sb, gxc)
            emit_gx_half(m, 1, xt_sb, gxc)

        for n in range(NCHK):
            gx_cur = gxc
            if n + 1 < NCHK:
                xt_sb = xp.tile([128, KE * SBc], BF16)
                nc.sync.dma_start(
                    xt_sb[:], xT[:, :, (n + 1) * SBc:(n + 2) * SBc])
                gxc = gxsp.tile([128, NM * SBc], BF16)

            for tt in range(CH):
                t = n * CH + tt

                def gx_ap(mlo, mn):
                    # contiguous [128, mn*b] slice of step tt's gx block
                    return gx_cur[:, tt * NM * b + mlo * b:
                                  tt * NM * b + (mlo + mn) * b]

                if t == 0:
                    sig = tp.tile([128, 3 * W], F32, tag="sig")
                    nc.scalar.activation(sig[:, 0:2 * W], gx_ap(0, 2 * NH),
                                         AFT.Sigmoid)
                    tg = tp.tile([128, W], F32, tag="tg")
                    nc.scalar.activation(tg[:], gx_ap(2 * NH, NH), AFT.Tanh)
                    nc.scalar.activation(sig[:, 2 * W:3 * W], gx_ap(3 * NH, NH),
                                         AFT.Sigmoid)
                    nc.vector.tensor_mul(c_sb[:], sig[:, W:2 * W], tg[:])
                else:
                    h_prev = hist[:, (t - 1) * W:t * W]

                    # own PSUM bank per gate block: start=True zeroing is
                    # bank-granular, so blocks must not share banks with
                    # regions still being read by the activations.
                    # The identity gx-adds go FIRST (start=True) — no h
                    # dependency, so they run during the previous step's
                    # activation tail while the PE is otherwise idle.
                    psfi = rp.tile([128, 2 * W], F32, tag="psfi")
                    psg = rp.tile([128, W], F32, tag="psg")
                    pso = rp.tile([128, W], F32, tag="pso")
                    for pst, mlo, mn in ((psfi, 0, 2 * NH), (psg, 2 * NH, NH),
                                         (pso, 3 * NH, NH)):
                        nc.tensor.matmul(
                            pst[:, 0:mn * b], ident_sb[:], gx_ap(mlo, mn),
                            start=True, stop=False, skip_group_check=True)

                    def whh_block(pst, mlo, mn):
                        for mi in range(mn):
                            mm = mlo + mi
                            for kt in range(NH):
                                nc.tensor.matmul(
                                    pst[:, mi * b:(mi + 1) * b],
                                    whh_sb[:, kt * 4 * HD + mm * 128:
                                           kt * 4 * HD + (mm + 1) * 128],
                                    h_prev[:, kt * b:(kt + 1) * b],
                                    start=False, stop=(kt == NH - 1),
                                    skip_group_check=True)

                    whh_block(psfi, 0, 2 * NH)    # f, i
                    sig = tp.tile([128, 3 * W], F32, tag="sig")
                    nc.scalar.activation(sig[:, 0:2 * W], psfi[:], AFT.Sigmoid)
                    whh_block(psg, 2 * NH, NH)    # g
                    tg = tp.tile([128, W], F32, tag="tg")
                    nc.scalar.activation(tg[:], psg[:], AFT.Tanh)
                    whh_block(pso, 3 * NH, NH)    # o
                    t1 = tp.tile([128, W], F32, tag="t1")
                    nc.vector.tensor_mul(t1[:], sig[:, 0:W], c_sb[:])
                    t2 = tp.tile([128, W], F32, tag="t2")
                    nc.vector.tensor_mul(t2[:], sig[:, W:2 * W], tg[:])
                    nc.vector.tensor_add(c_sb[:], t1[:], t2[:])
                    nc.scalar.activation(sig[:, 2 * W:3 * W], pso[:],
                                         AFT.Sigmoid)
                tanc = tp.tile([128, W], F32, tag="tanc")
                nc.scalar.activation(tanc[:], c_sb[:], AFT.Tanh)
                nc.vector.tensor_mul(hist[:, t * W:(t + 1) * W],
                                     sig[:, 2 * W:3 * W], tanc[:])

                # interleave GX production for chunk n+1 into this chunk
                if n + 1 < NCHK and tt // 2 < NM:
                    emit_gx_half(tt // 2, tt % 2, xt_sb, gxc)

            # ---- em partial for chunk n ----
            hv = hist[:, n * CH * W:(n + 1) * CH * W].rearrange(
                "p (t k c) -> p t k c", t=CH, k=NH)
            pse = ep.tile([T, SBc], F32)
            for kt in range(NH):
                nc.tensor.matmul(
                    pse[:].rearrange("p (t c) -> p t c", t=CH),
                    wtag_sb[:, kt * T:(kt + 1) * T],
                    hv[:, :, kt, :],
                    start=(kt == 0), stop=(kt == NH - 1))
            emin, emout, c0, c1 = next(s for s in emseg if s[2] <= n < s[3])
            noff = n - c0
            s0 = sp2.tile([T, SBc], BF16, tag="s0")
            nc.vector.tensor_scalar(s0[:], pse[:], tagb_sb[:], m0_sb[:],
                                    op0=ALU.add, op1=ALU.mult)
            nc.sync.dma_start(emin[0, :, noff * SBc:(noff + 1) * SBc], s0[:])
            s1 = sp2.tile([T, SBc], BF16, tag="s1")
            nc.vector.tensor_scalar(s1[:], pse[:], tagb_sb[:], m1_sb[:],
                                    op0=ALU.add, op1=ALU.mult)
            nc.sync.dma_start(emin[1, :, noff * SBc:(noff + 1) * SBc], s1[:])
            if n == c1 - 1:
                # segment em AllReduce overlaps the remaining recurrence
                nc.gpsimd.collective_compute(
                    "AllReduce", ALU.add,
                    replica_groups=[[c, c + NPAIR] for c in range(NPAIR)],
                    ins=[emin.opt()], outs=[emout.opt()])

        if stop_after == 'B':
            with tc.tile_pool(name="bail", bufs=1) as bp:
                bt = bp.tile([1, 1], F32)
                nc.vector.tensor_copy(bt[:], c_sb[0:1, 0:1])
                nc.sync.dma_start(loss[:], bt[:])
            ab.close()
            nc.compile()
            return nc

        ab.close()

        crf = top.enter_context(tc.tile_pool(name="crf", bufs=1))
        em_full = crf.tile([T, SB], F32, tag="emfull")
        eem = crf.tile([T, SB], F32, tag="eem")
        e0b = crf.tile([T, SB], BF16, tag="e0b")
        e1b = crf.tile([T, SB], BF16, tag="e1b")
        for _emin, emout, c0, c1 in emseg:
            col0, col1 = c0 * SBc, c1 * SBc
            nc.sync.dma_start(e0b[:, col0:col1], emout[0])
            # bwd slot is time-reversed: storage t' in [c0*CH, c1*CH) covers
            # real t = S-1-t', i.e. cols [SB-col1, SB-col0), reversed
            nc.sync.dma_start(
                e1b[:, SB - col1:SB - col0],
                emout[1].rearrange("j (t c) -> j t c",
                                   t=(c1 - c0) * CH)[:, ::-1, :])
        nc.vector.tensor_add(em_full[:], e0b[:], e1b[:])

        if stop_after == 'C':
            with tc.tile_pool(name="bail", bufs=1) as bp:
                bt = bp.tile([1, 1], F32)
                nc.vector.tensor_copy(bt[:], em_full[0:1, 0:1])
                nc.sync.dma_start(loss[:], bt[:])
            nc.compile()
            return nc

        # ---------------- Phase D: CRF ----------------
        with ExitStack() as ph:
            sp = ph.enter_context(tc.tile_pool(name="crftmp", bufs=2))
            big = ph.enter_context(tc.tile_pool(name="crfbig", bufs=2))
            ap_ = ph.enter_context(tc.tile_pool(name="alphas", bufs=2))

            cst = sp.tile([T, T], F32, tag="cst")        # transitions
            nc.sync.dma_start(cst[:], transm[:])
            st_sb = sp.tile([T, 1], F32, tag="stv")
            nc.sync.dma_start(st_sb[:], startv[:])
            en_sb = sp.tile([T, 1], F32, tag="env")
            nc.sync.dma_start(en_sb[:], endv[:])

            # --- numerator ---
            lab9 = big.tile([T, SB], I32, tag="big")
            nc.sync.dma_start(
                lab9[:],
                labT[:].rearrange("s c -> (s c)")[None, :].broadcast_to((T, SB)))
            labf = big.tile([T, SB], F32, tag="big")
            nc.vector.tensor_copy(labf[:], lab9[:])
            io9 = sp.tile([T, 1], I32, tag="io9")
            nc.gpsimd.iota(io9[:], pattern=[[0, 1]], base=0, channel_multiplier=1)
            io9f = sp.tile([T, 1], F32, tag="io9f")
            nc.vector.tensor_copy(io9f[:], io9[:])
            onehot = big.tile([T, SB], F32, tag="big")
            nc.vector.tensor_scalar(onehot[:], labf[:], io9f[:], None,
                                    op0=ALU.is_equal)
            gmul = big.tile([T, SB], F32, tag="big")
            nc.vector.tensor_mul(gmul[:], onehot[:], em_full[:])
            acc = sp.tile([T, b], F32, tag="acc")
            nc.vector.tensor_reduce(
                acc[:], gmul[:].rearrange("j (t c) -> j c t", c=b),
                op=ALU.add, axis=AXL.X)
            # start/end gold scores
            stsc = sp.tile([T, b], F32, tag="stsc")
            nc.vector.tensor_scalar_mul(stsc[:], onehot[:, 0:b], st_sb[:])
            nc.vector.tensor_add(acc[:], acc[:], stsc[:])
            ensc = sp.tile([T, b], F32, tag="ensc")
            nc.vector.tensor_scalar_mul(ensc[:], onehot[:, (S - 1) * b:S * b],
                                        en_sb[:])
            nc.vector.tensor_add(acc[:], acc[:], ensc[:])
            # transition gold scores: TH = T^T @ onehot ; V = TH * onehot_next
            numps = ExitStack()
            pp = numps.enter_context(
                tc.tile_pool(name="numps", bufs=2, space="PSUM"))
            for tc0 in range(0, S - 1, 32):
                tn = min(32, S - 1 - tc0)
                thp = pp.tile([T, 32 * b], F32, tag="thp")
                nc.tensor.matmul(thp[:, 0:tn * b], cst[:],
                                 onehot[:, tc0 * b:(tc0 + tn) * b],
                                 start=True, stop=True)
                v = sp.tile([T, 32 * b], F32, tag="v")
                nc.vector.tensor_mul(v[:, 0:tn * b], thp[:, 0:tn * b],
                                     onehot[:, (tc0 + 1) * b:(tc0 + 1 + tn) * b])
                vr = sp.tile([T, b], F32, tag="vr")
                nc.vector.tensor_reduce(
                    vr[:], v[:, 0:tn * b].rearrange("j (t c) -> j c t", c=b),
                    op=ALU.add, axis=AXL.X)
                nc.vector.tensor_add(acc[:], acc[:], vr[:])
            ones9 = sp.tile([T, 1], F32, tag="ones9")
            nc.vector.memset(ones9[:], 1.0)
            ones19 = sp.tile([1, T], F32, tag="ones19")
            nc.vector.memset(ones19[:], 1.0)
            nump = pp.tile([1, b], F32, tag="nump")
            nc.tensor.matmul(nump[:], ones9[:], acc[:], start=True, stop=True)
            num_sb = sp.tile([1, b], F32, tag="num")
            nc.vector.tensor_copy(num_sb[:], nump[:])
            numps.close()
            pp = ph.enter_context(tc.tile_pool(name="scanps", bufs=2, space="PSUM"))
            pp2 = ph.enter_context(tc.tile_pool(name="scanps2", bufs=2, space="PSUM"))

            # --- partition function (probability-domain scan, prescaled) ---
            NG = 2                     # interleaved batch groups
            gb = b // NG               # 8 sequences per group
            Em = sp.tile([T, T], F32, tag="Em")
            nc.scalar.activation(Em[:], cst[:], AFT.Exp)
            kneg = sp.tile([T, 1], F32, tag="kneg")
            nc.vector.memset(kneg[:], -KAPPA)
            nc.scalar.activation(eem[:], em_full[:], AFT.Exp, bias=kneg[:])
            es = sp.tile([T, 1], F32, tag="es")
            nc.scalar.activation(es[:], st_sb[:], AFT.Exp)
            ee = sp.tile([T, 1], F32, tag="ee")
            nc.scalar.activation(ee[:], en_sb[:], AFT.Exp)
            logacc = sp.tile([1, b], F32, tag="logacc")
            nc.vector.memset(logacc[:], 0.0)
            alphas = []
            for g in range(NG):
                al = ap_.tile([T, gb], F32, tag=f"al{g}")
                nc.vector.tensor_scalar_mul(al[:], eem[:, g * gb:(g + 1) * gb],
                                            es[:])
                alphas.append(al)
            for t in range(1, S):
                apss = []
                for g in range(NG):
                    aps = pp.tile([T, gb], F32, tag=f"aps{g}")
                    nc.tensor.matmul(aps[:], Em[:], alphas[g][:],
                                     start=True, stop=True)
                    apss.append(aps)
                for g in range(NG):
                    al = ap_.tile([T, gb], F32, tag=f"al{g}")
                    nc.vector.tensor_mul(
                        al[:], apss[g][:],
                        eem[:, t * b + g * gb:t * b + (g + 1) * gb])
                    alphas[g] = al
                if t % R == 0:
                    for g in range(NG):
                        ssum = pp2.tile([1, gb], F32, tag="ssum")
                        nc.tensor.matmul(ssum[:], ones9[:], alphas[g][:],
                                         start=True, stop=True)
                        ls = sp.tile([1, gb], F32, tag=f"ls{g}")
                        nc.scalar.activation(ls[:], ssum[:], AFT.Ln)
                        nc.vector.tensor_add(
                            logacc[:, g * gb:(g + 1) * gb],
                            logacc[:, g * gb:(g + 1) * gb], ls[:])
                        rc = sp.tile([1, gb], F32, tag=f"rc{g}")
                        nc.vector.reciprocal(rc[:], ssum[:])
                        bc = pp2.tile([T, gb], F32, tag="bc")
                        nc.tensor.matmul(bc[:], ones19[:], rc[:],
                                         start=True, stop=True)
                        al = ap_.tile([T, gb], F32, tag=f"al{g}")
                        nc.vector.tensor_mul(al[:], alphas[g][:], bc[:])
                        alphas[g] = al
            lv = sp.tile([1, b], F32, tag="lv")
            for g in range(NG):
                zp = pp.tile([1, gb], F32, tag=f"aps{g}")
                nc.tensor.matmul(zp[:], ee[:], alphas[g][:],
                                 start=True, stop=True)
                lz = sp.tile([1, gb], F32, tag=f"lz{g}")
                nc.scalar.activation(lz[:], zp[:], AFT.Ln)
                logz = sp.tile([1, gb], F32, tag=f"logz{g}")
                nc.vector.tensor_add(logz[:], lz[:],
                                     logacc[:, g * gb:(g + 1) * gb])
                # num - (logz + S*kappa)
                nc.vector.tensor_sub(lv[:, g * gb:(g + 1) * gb],
                                     num_sb[:, g * gb:(g + 1) * gb], logz[:])
            lvk = sp.tile([1, b], F32, tag="lvk")
            nc.vector.tensor_scalar_add(lvk[:], lv[:], -float(S) * KAPPA)
            tot = sp.tile([1, 1], F32, tag="tot")
            nc.vector.tensor_reduce(tot[:], lvk[:], op=ALU.add, axis=AXL.X)
            sc = sp.tile([1, 1], F32, tag="sc")
            nc.vector.tensor_scalar_mul(sc[:], tot[:], -1.0 / (2.0 * B_full))
            nc.sync.dma_start(lossdb[:], sc[:])
            nc.gpsimd.collective_compute(
                "AllReduce", ALU.add,
                replica_groups=[list(range(NCORES))],
                ins=[lossdb.opt()], outs=[lossout.opt()])
            lf = sp.tile([1, 1], F32, tag="lf")
            nc.sync.dma_start(lf[:], lossout[:])
            nc.sync.dma_start(loss[:], lf[:])

    nc.compile()
    return nc


# ---------------------------------------------------------------------------
# host-side sharding
# ---------------------------------------------------------------------------

def _perm_figo(HD):
    # torch gate order i,f,g,o -> f,i,g,o
    return np.concatenate([
        np.arange(HD, 2 * HD), np.arange(0, HD),
        np.arange(2 * HD, 3 * HD), np.arange(3 * HD, 4 * HD)])


def shard_inputs(inputs, b, S, E, HD, T):
    KE, NH = E // 128, HD // 128
    perm = _perm_figo(HD)
    bf = ml_dtypes.bfloat16
    x = np.asarray(inputs["x"], np.float32)
    labels = np.asarray(inputs["labels"]).astype(np.int32)
    trans = np.asarray(inputs["transitions"], np.float32)
    startv = np.asarray(inputs["start_trans"], np.float32).reshape(T, 1)
    endv = np.asarray(inputs["end_trans"], np.float32).reshape(T, 1)
    Wtag = np.asarray(inputs["W_tag"], np.float32)
    btag = np.asarray(inputs["b_tag"], np.float32).reshape(T, 1)
    identm = np.eye(128, dtype=np.float32).astype(bf)

    per_dir = {}
    for d, sfx in enumerate(("f", "b")):
        Wih = np.asarray(inputs[f"W_ih_{sfx}"], np.float32)[perm]
        Whh = np.asarray(inputs[f"W_hh_{sfx}"], np.float32)[perm]
        bias = (np.asarray(inputs[f"b_ih_{sfx}"], np.float32)
                + np.asarray(inputs[f"b_hh_{sfx}"], np.float32))[perm]
        per_dir[d] = dict(
            wihT=np.ascontiguousarray(
                Wih.T.reshape(KE, 128, 4 * HD)).astype(bf),
            whhT=np.ascontiguousarray(
                Whh.T.reshape(NH, 128, 4 * HD)).astype(bf),
            bias4=np.ascontiguousarray(
                bias.reshape(4 * NH, 128).T).astype(np.float32),
            wtagT=np.ascontiguousarray(
                Wtag[:, d * HD:(d + 1) * HD].T.reshape(NH, 128, T)).astype(bf),
            tagb=btag if d == 0 else np.zeros_like(btag),
            m0=np.full((T, 1), 1.0 - d, np.float32),
            m1=np.full((T, 1), float(d), np.float32),
        )

    in_maps = []
    for c in range(NCORES):
        d = c // NPAIR                      # 0 fwd, 1 bwd
        g = c % NPAIR                       # batch group
        xs = x[g * b:(g + 1) * b]           # (b, S, E)
        if d == 1:
            xs = xs[:, ::-1, :]
        xTc = np.ascontiguousarray(xs.transpose(2, 1, 0).reshape(KE, 128, S * b)
                                   ).astype(bf)
        m = dict(per_dir[d])
        m["xT"] = xTc
        m["labT"] = np.ascontiguousarray(labels[g * b:(g + 1) * b].T)
        m["transm"] = trans
        m["startv"] = startv
        m["endv"] = endv
        m["ident"] = identm
        in_maps.append(m)
    return in_maps


# ---------------------------------------------------------------------------
# entry point
# ---------------------------------------------------------------------------

_B, _S, _E, _HD, _T = 64, 512, 1024, 512, 9
_cache = {}


def _get_program():
    if "nc" not in _cache:
        _cache["nc"] = build_program(_B // NPAIR, _S, _E, _HD, _T, _B)
    return _cache["nc"]


def kernel(**inputs) -> np.ndarray:
    from concourse.bass_utils import run_bass_kernel_spmd
    nc = _get_program()
    in_maps = shard_inputs(inputs, _B // NPAIR, _S, _E, _HD, _T)
    res = run_bass_kernel_spmd(nc, in_maps, list(range(NCORES)))
    out = np.asarray(res.results[0]["loss"], np.float32).reshape(())
    return out
